# revision 40
# baseline (speedup 1.0000x reference)
"""Trainium2 Bass kernel for nn_Decoder_86921548137026.

Dynamic decoder: NITER=2 iterations of (maxout pointer scoring over L=1024
positions -> argmax -> gather -> LSTM cell), followed by log_softmax over the
final start/end scores.

Sharding: data-parallel over batch B=32 across 8 cores (4 batches/core),
weights replicated.

v2 changes vs baseline:
  - serial path (r = tanh(cat@wd), o = r@w1r+b1, LSTM, mlp) flipped to row
    layout [4, n] with fp16 weights: ~10 small fp16 matmuls instead of ~20
    1us fp32 column matmuls (validated offline: rel 3.1e-4 end to end)
  - biases folded as ones-rows in the stationary operand (b1/b_lstm/b_mlp)
  - argmax gather: one dynamic column DMA per (batch, chunkset) dispatched
    round-robin across engine queues instead of 16 serialized sync DMAs
  - S4 row assembly via PE transpose + single DMA per batch (was 8 column
    DMAs per batch)
  - E cast f32->fp16 on vector/scalar engines (was gpsimd, 5x slower), with
    batch-0-first ordering and early sweep start; weight DMAs ordered so the
    first sweep only waits on its own weights
"""

import numpy as np

H = 200
PMX = 8
B = 32
L = 1024
BIG = np.float32(1e30)
NCORES = 8
BLOC = B // NCORES          # 4 batches per core
NLT = L // 128              # 8 l-tiles per batch
# stage-1/2 output channels = H*P = 1600, swept in PSUM-bank-sized chunks
NCH = [(0, 512), (512, 512), (1024, 512), (1536, 64)]
# h-slice of m1/m2 produced by each n-chunk (1600 = 200h * 8p, h-major)
HSL = [(0, 64), (64, 64), (128, 64), (192, 8)]

_cache = {}


def _build_program():
    import contextlib
    import concourse.mybir as mybir
    import concourse.tile as tile
    from concourse import bacc
    from concourse.bass import ds
    from concourse.masks import make_identity

    f32 = mybir.dt.float32
    f16 = mybir.dt.float16
    u32 = mybir.dt.uint32
    AF = mybir.ActivationFunctionType
    OP = mybir.AluOpType
    AX = mybir.AxisListType

    nc = bacc.Bacc("TRN2", target_bir_lowering=False, debug=False,
                   enable_asserts=False, num_devices=NCORES)

    # ---------------- DRAM I/O ----------------
    enc = nc.dram_tensor("enc", [BLOC, 2 * H, L], f32, kind="ExternalInput").ap()
    pen = nc.dram_tensor("pen", [BLOC, L], f32, kind="ExternalInput").ap()
    wt = {}
    for tag in ("s", "e"):
        wt[f"w1_{tag}"] = nc.dram_tensor(f"w1_{tag}", [3 * H, H * PMX], f32, kind="ExternalInput").ap()
        wt[f"b1_{tag}"] = nc.dram_tensor(f"b1_{tag}", [1, H * PMX], f32, kind="ExternalInput").ap()
        wt[f"w2_{tag}"] = nc.dram_tensor(f"w2_{tag}", [H, H * PMX], f32, kind="ExternalInput").ap()
        wt[f"b2h_{tag}"] = nc.dram_tensor(f"b2h_{tag}", [1, H * PMX], f16, kind="ExternalInput").ap()
        wt[f"b2l_{tag}"] = nc.dram_tensor(f"b2l_{tag}", [1, H * PMX], f16, kind="ExternalInput").ap()
        wt[f"w3_{tag}"] = nc.dram_tensor(f"w3_{tag}", [2 * H, PMX], f32, kind="ExternalInput").ap()
        wt[f"b3h_{tag}"] = nc.dram_tensor(f"b3h_{tag}", [1, PMX], f16, kind="ExternalInput").ap()
        wt[f"b3l_{tag}"] = nc.dram_tensor(f"b3l_{tag}", [1, PMX], f16, kind="ExternalInput").ap()
        wt[f"wd_{tag}"] = nc.dram_tensor(f"wd_{tag}", [5 * H, H], f32, kind="ExternalInput").ap()
    wt["w_ih"] = nc.dram_tensor("w_ih", [4 * H, 4 * H], f32, kind="ExternalInput").ap()
    wt["b_lstm"] = nc.dram_tensor("b_lstm", [1, 4 * H], f32, kind="ExternalInput").ap()
    wt["w_mlp"] = nc.dram_tensor("w_mlp", [H, H], f32, kind="ExternalInput").ap()
    wt["b_mlp"] = nc.dram_tensor("b_mlp", [1, H], f32, kind="ExternalInput").ap()

    lp1 = nc.dram_tensor("lp1", [BLOC, L], f32, kind="ExternalOutput").ap()
    lp2 = nc.dram_tensor("lp2", [BLOC, L], f32, kind="ExternalOutput").ap()

    with tile.TileContext(nc) as tc, contextlib.ExitStack() as ctx:
        const = ctx.enter_context(tc.tile_pool(name="const", bufs=1))
        wpool = ctx.enter_context(tc.tile_pool(name="wpool", bufs=1))
        epool = ctx.enter_context(tc.tile_pool(name="epool", bufs=1))
        work = ctx.enter_context(tc.tile_pool(name="work", bufs=3))
        single = ctx.enter_context(tc.tile_pool(name="single", bufs=1))
        keep = ctx.enter_context(tc.tile_pool(name="keep", bufs=1))
        ps_a = ctx.enter_context(tc.tile_pool(name="ps_a", bufs=3, space="PSUM"))
        ps_b = ctx.enter_context(tc.tile_pool(name="ps_b", bufs=2, space="PSUM"))
        ps_tr = ctx.enter_context(tc.tile_pool(name="ps_tr", bufs=1, space="PSUM"))
        ps_s3 = ctx.enter_context(tc.tile_pool(name="ps_s3", bufs=1, space="PSUM"))
        ps_d = ctx.enter_context(tc.tile_pool(name="ps_d", bufs=1, space="PSUM"))
        stage_cm = tc.tile_pool(name="stage", bufs=2)
        stage = stage_cm.__enter__()

        # ---------------- constants ----------------
        ident = const.tile([128, 128], f16, name="ident")
        make_identity(nc, ident)
        ident32 = const.tile([128, 128], f32, name="ident32")
        make_identity(nc, ident32)

        # penalty mask in row layout [BLOC, L]
        pen_row = const.tile([BLOC, L], f32, name="pen_row")
        nc.sync.dma_start(pen_row, pen)

        # ---------------- load + cast E (per batch, fp16) ----------------
        # big tile [128, 3, L] per batch (chunks c0..c2) + c3 [18, L] with
        # ones rows 16,17 (o-fold lhsT rows); one batched DMA per batch,
        # all casts on the vector engine (scalar is busy with s-weights)
        E16 = []
        E16c3 = []
        es16 = keep.tile([128, 3, BLOC], f16, name="es16")
        es16c3 = keep.tile([17, BLOC], f16, name="es16c3")
        nc.vector.memset(es16c3, 1.0)              # row 16 = b_lstm fold row
        ee16 = keep.tile([128, 3, BLOC], f16, name="ee16")
        ee16c3 = keep.tile([16, BLOC], f16, name="ee16c3")
        def load_E(b, defer=None):
            est = stage.tile([128, 3, L], f32, tag="estg", bufs=2)
            for c in range(3):
                nc.sync.dma_start(est[:, c, :], enc[b, 128 * c:128 * (c + 1), :])
            et = epool.tile([128, 3, L], f16, name=f"E16_{b}")
            if defer is None:
                nc.vector.tensor_copy(et, est)
            else:
                # cast deferred into sweep-s1 fill steps on the scalar queue
                # (keeps the pre-sweep vector queue clear for the reduces)
                defer.append(lambda et=et, est=est:
                             nc.scalar.activation(et, est, AF.Copy))
            E16.append(et)
            # initial es = E[:, 0, :] straight from the f32 stage (doesn't
            # wait on the big cast)
            nc.vector.tensor_copy(es16[:, :, b:b + 1], est[:, :, 0:1])
            est2 = stage.tile([128, 1600], f32, tag="stg")
            dma_eng2 = nc.scalar if b % 2 else nc.sync
            dma_eng2.dma_start(est2[:16, :L], enc[b, 384:400, :])
            ec3 = epool.tile([18, L], f16, name=f"E16c3_{b}")
            nc.vector.memset(ec3, 1.0)
            if defer is None:
                nc.vector.tensor_copy(ec3[:16], est2[:16, :L])
            else:
                defer.append(lambda ec3=ec3, est2=est2:
                             nc.scalar.activation(ec3[:16], est2[:16, :L], AF.Copy))
            nc.vector.tensor_copy(es16c3[:16, b:b + 1], est2[:16, 0:1])
            E16c3.append(ec3)

        # batches 0,1 first; 2,3 after the s-weights so the first sweep's
        # weight DMAs aren't starved behind all 6.5MB of E traffic
        load_E(0)
        load_E(1)

        # ---------------- weights ----------------
        # order: s-scoring weights first (first sweep waits only on these).
        # tag "s" casts: scalar (sweep weights) + vector (serial weights);
        # tag "e" + lstm casts: gpsimd (idle during sweep s1, done by gap 1)
        W = {}

        def cast(eng, dst, src):
            if eng is nc.scalar:
                nc.scalar.activation(dst, src, AF.Copy)
            else:
                eng.tensor_copy(dst, src)

        def load_scoring(tag, dma, csw, cse, stg):
            # generator: yields after each dma+cast step so the caller can
            # interleave emission with sweep instructions
            # stage-1 rhs rows 0..383 as fp16 [128, 3, 1600]
            w1m = wpool.tile([128, 3, H * PMX], f16, name=f"w1m_{tag}")
            for c in range(3):
                wst = stage.tile([128, 1600], f32, tag=stg, bufs=2)
                dma.dma_start(wst, wt[f"w1_{tag}"][128 * c:128 * (c + 1)])
                cast(csw, w1m[:, c, :], wst)
                yield
            W[f"w1m_{tag}"] = w1m
            # shared c3 rhs: rows 0..15 = W1[384:400] fp16, rows 16/17 = o hi/lo
            # (double-buffered by batch parity to relax WAR stalls)
            wst = stage.tile([128, 1600], f32, tag=stg, bufs=2)
            dma.dma_start(wst[:16], wt[f"w1_{tag}"][384:400])
            c3pair = []
            for pi in range(2):
                c3 = wpool.tile([18, H * PMX], f16, name=f"c3_{tag}_{pi}")
                cast(csw, c3[:16], wst[:16])
                c3pair.append(c3)
            yield
            W[f"c3_{tag}"] = c3pair
            # o-matmul rhs (fp16): rows 400..599 of w1, + b1 as ones-row 72 of c2
            wst = stage.tile([128, 1600], f32, tag=stg, bufs=2)
            dma.dma_start(wst, wt[f"w1_{tag}"][400:528])
            w1rc1 = wpool.tile([128, H * PMX], f16, name=f"w1r16c1_{tag}")
            cast(cse, w1rc1, wst)
            yield
            wst = stage.tile([128, 1600], f32, tag=stg, bufs=2)
            dma.dma_start(wst[:72], wt[f"w1_{tag}"][528:600])
            dma.dma_start(wst[72:73], wt[f"b1_{tag}"])
            w1rc2 = wpool.tile([73, H * PMX], f16, name=f"w1r16c2_{tag}")
            cast(cse, w1rc2, wst[:73])
            yield
            W[f"w1r_{tag}"] = (w1rc1, w1rc2)
            # wd fp16 k-chunk tiles over cat(hx[0:200], es[200:600], ee[600:1000])
            wd_chunks = []
            for (k0, kn) in [(0, 128), (128, 72), (200, 128), (328, 128), (456, 128), (584, 16),
                             (600, 128), (728, 128), (856, 128), (984, 16)]:
                wst = stage.tile([128, 1600], f32, tag=stg, bufs=2)
                dma.dma_start(wst[:kn, :H], wt[f"wd_{tag}"][k0:k0 + kn])
                t = wpool.tile([kn, H], f16, name=f"wd16_{tag}_{k0}")
                cast(cse, t, wst[:kn, :H])
                wd_chunks.append((k0, kn, t))
                yield
            W[f"wd_{tag}"] = wd_chunks

        def load_scoring2(tag, dma, csw, stg):
            # stage-2 rhs
            w2c1 = wpool.tile([128, H * PMX], f16, name=f"w2c1_{tag}")
            wst = stage.tile([128, 1600], f32, tag=stg, bufs=2)
            dma.dma_start(wst, wt[f"w2_{tag}"][:128])
            cast(csw, w2c1, wst)
            yield
            W[f"w2c1_{tag}"] = w2c1
            w2c2 = wpool.tile([74, H * PMX], f16, name=f"w2c2_{tag}")
            wst = stage.tile([128, 1600], f32, tag=stg, bufs=2)
            dma.dma_start(wst[:72], wt[f"w2_{tag}"][128:200])
            cast(csw, w2c2[:72], wst[:72])
            dma.dma_start(w2c2[72:73, :], wt[f"b2h_{tag}"])
            dma.dma_start(w2c2[73:74, :], wt[f"b2l_{tag}"])
            yield
            W[f"w2c2_{tag}"] = w2c2
            # stage-3 rhs chunks
            wst = stage.tile([128, 1600], f32, tag=stg, bufs=2)
            dma.dma_start(wst[:, 0:8], wt[f"w3_{tag}"][0:128])
            dma.dma_start(wst[:72, 8:16], wt[f"w3_{tag}"][128:200])
            dma.dma_start(wst[:, 16:24], wt[f"w3_{tag}"][200:328])
            dma.dma_start(wst[:72, 24:32], wt[f"w3_{tag}"][328:400])
            w3c1 = wpool.tile([128, PMX], f16, name=f"w3c1_{tag}")
            cast(csw, w3c1, wst[:, 0:8])
            w3c2 = wpool.tile([74, PMX], f16, name=f"w3c2_{tag}")
            cast(csw, w3c2[:72], wst[:72, 8:16])
            yield
            dma.dma_start(w3c2[72:73, :], wt[f"b3h_{tag}"])
            dma.dma_start(w3c2[73:74, :], wt[f"b3l_{tag}"])
            w3c3 = wpool.tile([128, PMX], f16, name=f"w3c3_{tag}")
            cast(csw, w3c3, wst[:, 16:24])
            w3c4 = wpool.tile([72, PMX], f16, name=f"w3c4_{tag}")
            cast(csw, w3c4, wst[:72, 24:32])
            yield
            W[f"w3_{tag}"] = (w3c1, w3c2, w3c3, w3c4)

        def load_lstm(dma, cse):
            # LSTM weights as fp16 rhs [kn, 800] over rows = cat(es, ee);
            # es-c3 chunk carries b_lstm as ones-row 16
            ih_chunks = []
            for (k0, kn) in [(0, 128), (128, 128), (256, 128), (384, 16),
                             (400, 128), (528, 128), (656, 128), (784, 16)]:
                wst = stage.tile([128, 1600], f32, tag="wstg_e", bufs=2)
                dma.dma_start(wst[:kn, :800], wt["w_ih"][k0:k0 + kn])
                rows = kn + 1 if k0 == 384 else kn
                t = wpool.tile([rows, 4 * H], f16, name=f"wih16_{k0}")
                if k0 == 384:
                    dma.dma_start(wst[16:17, :800], wt["b_lstm"])
                cast(cse, t[:rows], wst[:rows, :800])
                ih_chunks.append((k0, kn, t))
                yield
            # mlp fp16 rhs [kn, 200]; c2 carries b_mlp as ones-row 72
            wst = stage.tile([128, 1600], f32, tag="wstg_e", bufs=2)
            dma.dma_start(wst[:, :H], wt["w_mlp"][0:128])
            wmlpc1 = wpool.tile([128, H], f16, name="wmlp16c1")
            cast(cse, wmlpc1, wst[:, :H])
            yield
            wst = stage.tile([128, 1600], f32, tag="wstg_e", bufs=2)
            dma.dma_start(wst[:72, :H], wt["w_mlp"][128:200])
            dma.dma_start(wst[72:73, :H], wt["b_mlp"])
            wmlpc2 = wpool.tile([73, H], f16, name="wmlp16c2")
            cast(cse, wmlpc2, wst[:73, :H])
            W["ih_chunks"] = ih_chunks
            W["wmlp"] = (wmlpc1, wmlpc2)
            yield

        # s-tag weights emitted eagerly (first sweep waits on them); e-tag +
        # lstm weights emitted as fill steps interleaved into sweep s1
        import itertools
        for _ in load_scoring("s", nc.sync, nc.scalar, nc.vector, "stg"):
            pass
        for _ in load_scoring2("s", nc.sync, nc.scalar, "stg"):
            pass
        edefer = []
        load_E(2, edefer)
        load_E(3, edefer)

        def edefer_gen():
            for fn in edefer:
                fn()
                yield
        fill_steps = itertools.chain(
            edefer_gen(),
            load_scoring("e", nc.sync, nc.scalar, nc.scalar, "wstg_e"),
            load_scoring2("e", nc.sync, nc.scalar, "wstg_e"),
            load_lstm(nc.sync, nc.scalar))

        # ---------------- persistent state tiles ----------------
        hxT = [keep.tile([128, BLOC], f16, name="hxT16_0"),
               keep.tile([72, BLOC], f16, name="hxT16_1")]
        rT1 = keep.tile([128, BLOC], f16, name="rT1")
        rT2 = keep.tile([73, BLOC], f16, name="rT2")
        nc.gpsimd.memset(rT2, 1.0)                 # row 72 = b1 fold row
        h0T1 = keep.tile([128, BLOC], f16, name="h0T1")
        h0T2 = keep.tile([73, BLOC], f16, name="h0T2")
        nc.gpsimd.memset(h0T2, 1.0)                # row 72 = b_mlp fold row

        # es/ee init happened inline in the E load loop (from the f32 stage)
        nc.vector.tensor_copy(ee16, es16)
        nc.vector.tensor_copy(ee16c3, es16c3[:16])

        # manually rotated work slots; m1c2 slots carry persistent ones rows
        # 72/73 (paired with the b2/b3 hi+lo rhs rows)
        m1_slots, m2_slots, m1c2_slots = [], [], []
        for i in range(4):
            m1_slots.append(keep.tile([128, H], f16, name=f"m1_slot{i}"))
            m2_slots.append(keep.tile([128, H], f16, name=f"m2_slot{i}"))
            t = keep.tile([74, 128], f16, name=f"m1c2_slot{i}")
            nc.gpsimd.memset(t, 1.0)
            m1c2_slots.append(t)
        S4_a = keep.tile([BLOC, L], f32, name="S4_a")
        S4_b = keep.tile([BLOC, L], f32, name="S4_b")

        dma_engines = [nc.sync, nc.scalar, nc.gpsimd]

        # ---------------- helpers ----------------
        def cat_chunks(tag, with_hx):
            """(lhsT [kn,4] fp16, wd16 [kn,H]) pairs for r = tanh(cat @ wd)."""
            ops = []
            for (k0, kn, wtile) in W[f"wd_{tag}"]:
                if k0 < 200:
                    if not with_hx:
                        continue
                    lhsT = hxT[0] if k0 == 0 else hxT[1]
                elif k0 < 600:
                    c = (k0 - 200) // 128
                    lhsT = es16[:, c, :] if c < 3 else es16c3[:16]
                else:
                    c = (k0 - 600) // 128
                    lhsT = ee16[:, c, :] if c < 3 else ee16c3
                ops.append((lhsT, wtile))
            return ops

        def r_matmul(tag, with_hx):
            """r_row = tanh(cat @ wd) -> [4, H] fp16 sbuf."""
            ops = cat_chunks(tag, with_hx)
            pt = ps_d.tile([128, 512], f32, tag="ps_ser")
            for i, (lhsT, rhs) in enumerate(ops):
                nc.tensor.matmul(pt[:BLOC, :H], lhsT, rhs,
                                 start=(i == 0), stop=(i == len(ops) - 1))
            r_row = work.tile([BLOC, H], f16, tag="r_row", bufs=1)
            nc.scalar.activation(r_row, pt[:BLOC, :H], AF.Tanh)
            return r_row

        def o_rows(tag, r_row):
            """o = r@w1r + b1 (f32 psum) -> fp16 hi/lo rows [BLOC, 1600]."""
            # transpose r to column chunks [128,4], [72,4] (+ones row 72)
            ptr = ps_tr.tile([128, 128], f16, tag="ps_tr")
            nc.tensor.transpose(ptr[:, :BLOC], r_row[:, 0:128], ident[:BLOC, :BLOC])
            nc.vector.tensor_copy(rT1, ptr[:, :BLOC])
            ptr2 = ps_tr.tile([128, 128], f16, tag="ps_tr")
            nc.tensor.transpose(ptr2[:72, :BLOC], r_row[:, 128:200], ident[:BLOC, :BLOC])
            nc.vector.tensor_copy(rT2[:72], ptr2[:72, :BLOC])
            w1rc1, w1rc2 = W[f"w1r_{tag}"]
            oh = single.tile([BLOC, H * PMX], f16, tag="oh")
            ol = single.tile([BLOC, H * PMX], f16, tag="ol")
            for (n0, nn) in NCH:
                pt = ps_d.tile([128, 512], f32, tag="ps_ser")
                nc.tensor.matmul(pt[:BLOC, :nn], rT1, w1rc1[:, n0:n0 + nn], start=True, stop=False)
                nc.tensor.matmul(pt[:BLOC, :nn], rT2, w1rc2[:, n0:n0 + nn], start=False, stop=True)
                nc.scalar.activation(oh[:, n0:n0 + nn], pt[:BLOC, :nn], AF.Copy)
                nc.vector.tensor_tensor(ol[:, n0:n0 + nn], pt[:BLOC, :nn], oh[:, n0:n0 + nn], OP.subtract)
            return oh, ol

        def score_sweep(tag, S4, oh, ol, fill=None):
            """Maxout scoring, software-pipelined across (b, lt) tiles."""
            w1m = W[f"w1m_{tag}"]
            c3pair = W[f"c3_{tag}"]
            w2c1 = W[f"w2c1_{tag}"]
            w2c2 = W[f"w2c2_{tag}"]
            w3c1, w3c2, w3c3, w3c4 = W[f"w3_{tag}"]
            NT = BLOC * NLT
            st = [dict() for _ in range(NT)]
            strips = {}

            def g1(i):
                b, lt = divmod(i, NLT)
                c3rhs = c3pair[b % 2]
                if lt == 0:
                    nc.sync.dma_start(c3rhs[16:17, :], oh[b:b + 1, :])
                    nc.sync.dma_start(c3rhs[17:18, :], ol[b:b + 1, :])
                lsl = slice(128 * lt, 128 * (lt + 1))
                m1 = m1_slots[i % 4]
                for ni, (n0, nn) in enumerate(NCH):
                    pa = ps_a.tile([128, 512], f32, tag="ps_s1")
                    for c in range(3):
                        nc.tensor.matmul(pa[:, :nn], E16[b][:, c, lsl], w1m[:, c, n0:n0 + nn],
                                         start=(c == 0), stop=False)
                    nc.tensor.matmul(pa[:, :nn], E16c3[b][:, lsl], c3rhs[:, n0:n0 + nn],
                                     start=False, stop=True)
                    h0, hn = HSL[ni]
                    nc.vector.tensor_reduce(
                        m1[:, h0:h0 + hn],
                        pa[:, :nn].rearrange("p (h q) -> p h q", q=PMX),
                        axis=AX.X, op=OP.max)
                st[i]["m1"] = m1

            def g2(i):
                m1 = st[i]["m1"]
                pt1 = ps_tr.tile([128, 128], f16, tag="ps_tr")
                nc.tensor.transpose(pt1, m1[:, 0:128], ident)
                m1c1 = work.tile([128, 128], f16, tag="m1c1")
                nc.scalar.activation(m1c1, pt1, AF.Copy)
                pt2 = ps_tr.tile([128, 128], f16, tag="ps_tr")
                nc.tensor.transpose(pt2[:72], m1[:, 128:200], ident)
                m1c2 = m1c2_slots[i % 4]
                nc.scalar.activation(m1c2[:72], pt2[:72], AF.Copy)
                m2 = m2_slots[i % 4]
                for ni, (n0, nn) in enumerate(NCH):
                    pb = ps_b.tile([128, 512], f32, tag="ps_s2")
                    nc.tensor.matmul(pb[:, :nn], m1c1, w2c1[:, n0:n0 + nn], start=True, stop=False)
                    nc.tensor.matmul(pb[:, :nn], m1c2, w2c2[:, n0:n0 + nn], start=False, stop=True)
                    h0, hn = HSL[ni]
                    nc.vector.tensor_reduce(
                        m2[:, h0:h0 + hn],
                        pb[:, :nn].rearrange("p (h q) -> p h q", q=PMX),
                        axis=AX.X, op=OP.max)
                st[i]["m1c1"] = m1c1
                st[i]["m1c2"] = m1c2
                st[i]["m2"] = m2

            def g3(i):
                b, lt = divmod(i, NLT)
                m2 = st[i]["m2"]
                pt3 = ps_tr.tile([128, 128], f16, tag="ps_tr")
                nc.tensor.transpose(pt3, m2[:, 0:128], ident)
                m2c1 = work.tile([128, 128], f16, tag="m2c1")
                nc.scalar.activation(m2c1, pt3, AF.Copy)
                pt4 = ps_tr.tile([128, 128], f16, tag="ps_tr")
                nc.tensor.transpose(pt4[:72], m2[:, 128:200], ident)
                m2c2 = work.tile([72, 128], f16, tag="m2c2")
                nc.scalar.activation(m2c2, pt4[:72], AF.Copy)
                if lt == 0:
                    strips[b] = ps_s3.tile([128, 8 * NLT], f32, tag="ps_s3", name="s3strip")
                psl = strips[b][:, 8 * lt:8 * (lt + 1)]
                nc.tensor.matmul(psl, st[i]["m1c1"], w3c1, start=True, stop=False)
                nc.tensor.matmul(psl, st[i]["m1c2"], w3c2, start=False, stop=False)
                nc.tensor.matmul(psl, m2c1, w3c3, start=False, stop=False)
                nc.tensor.matmul(psl, m2c2, w3c4, start=False, stop=True)
                st[i].clear()
                if lt == NLT - 1:
                    Sb = work.tile([128, NLT], f32, tag="Sb")
                    nc.vector.tensor_reduce(Sb,
                                            strips[b].rearrange("p (t q) -> p t q", q=PMX),
                                            axis=AX.X, op=OP.max)
                    ptb = ps_tr.tile([NLT, 128], f32, tag="ps_tr")
                    nc.tensor.transpose(ptb, Sb, ident32)
                    s4stg = work.tile([NLT, 128], f32, tag="s4stg")
                    nc.scalar.activation(s4stg, ptb, AF.Copy)
                    dma_engines[b % 3].dma_start(S4[b:b + 1, :], s4stg)
            for i in range(NT + 2):
                if i < NT:
                    g1(i)
                if 1 <= i < NT + 1:
                    g2(i - 1)
                if 2 <= i:
                    g3(i - 2)
                if fill is not None:
                    next(fill, None)
            if fill is not None:
                for _ in fill:
                    pass
            nc.vector.tensor_tensor(S4, S4, pen_row, OP.subtract)

        def argmax_gather(S4, dstbig, dstc3):
            """argmax over S4 rows; gather E columns (fp16)."""
            mx8 = work.tile([BLOC, 8], f32, tag="mx8")
            idx8 = work.tile([BLOC, 8], u32, tag="idx8")
            nc.vector.max(out=mx8, in_=S4)
            nc.vector.max_index(out=idx8, in_max=mx8, in_values=S4)
            for b in range(BLOC):
                reg = nc.values_load(idx8[b:b + 1, 0:1], min_val=0, max_val=L - 1,
                                     skip_runtime_bounds_check=True)
                dma_engines[(2 * b) % 3].dma_start(
                    dstbig[:, :, b:b + 1], E16[b][:, :, ds(reg, 1)])
                dma_engines[(2 * b + 1) % 3].dma_start(
                    dstc3[:16, b:b + 1], E16c3[b][:16, ds(reg, 1)])

        def lstm_update():
            """hx via LSTM cell with hx0=cx0=0 (f-gate and w_hh drop out)."""
            # gates row-layout: psum [4, 200] for i; [4, 400] for g,o
            pt_i = ps_d.tile([128, 512], f32, tag="ps_ser")
            pt_go = ps_s3.tile([128, 512], f32, tag="ps_s3")
            lhs_for = []
            for (k0, kn, wtile) in W["ih_chunks"]:
                if k0 < 400:
                    c = k0 // 128
                    lhsT = es16[:, c, :] if c < 3 else es16c3  # [17,4] w/ ones
                else:
                    c = (k0 - 400) // 128
                    lhsT = ee16[:, c, :] if c < 3 else ee16c3
                lhs_for.append((lhsT, wtile, kn + (1 if k0 == 384 else 0)))
            n = len(lhs_for)
            for i, (lhsT, wtile, rows) in enumerate(lhs_for):
                nc.tensor.matmul(pt_i[:BLOC, :H], lhsT, wtile[:rows, 0:H],
                                 start=(i == 0), stop=(i == n - 1))
            for i, (lhsT, wtile, rows) in enumerate(lhs_for):
                nc.tensor.matmul(pt_go[:BLOC, :2 * H], lhsT, wtile[:rows, 2 * H:4 * H],
                                 start=(i == 0), stop=(i == n - 1))
            ig = work.tile([BLOC, H], f32, tag="ig", bufs=1)
            nc.scalar.activation(ig, pt_i[:BLOC, :H], AF.Sigmoid)
            gg = work.tile([BLOC, H], f32, tag="gg", bufs=1)
            nc.scalar.activation(gg, pt_go[:BLOC, 0:H], AF.Tanh)
            og = work.tile([BLOC, H], f32, tag="og", bufs=1)
            nc.scalar.activation(og, pt_go[:BLOC, H:2 * H], AF.Sigmoid)
            cx = work.tile([BLOC, H], f32, tag="cx", bufs=1)
            nc.vector.tensor_tensor(cx, ig, gg, OP.mult)
            tcx = work.tile([BLOC, H], f32, tag="tcx", bufs=1)
            nc.scalar.activation(tcx, cx, AF.Tanh)
            h0 = work.tile([BLOC, H], f16, tag="h0", bufs=1)
            nc.vector.tensor_tensor(h0, og, tcx, OP.mult)
            # transpose h0 -> column chunks (+ones row 72 for b_mlp)
            ptr = ps_tr.tile([128, 128], f16, tag="ps_tr")
            nc.tensor.transpose(ptr[:, :BLOC], h0[:, 0:128], ident[:BLOC, :BLOC])
            nc.vector.tensor_copy(h0T1, ptr[:, :BLOC])
            ptr2 = ps_tr.tile([128, 128], f16, tag="ps_tr")
            nc.tensor.transpose(ptr2[:72, :BLOC], h0[:, 128:200], ident[:BLOC, :BLOC])
            nc.vector.tensor_copy(h0T2[:72], ptr2[:72, :BLOC])
            # mlp: hx = h0 @ w_mlp + b_mlp
            pt = ps_d.tile([128, 512], f32, tag="ps_ser")
            wmlpc1, wmlpc2 = W["wmlp"]
            nc.tensor.matmul(pt[:BLOC, :H], h0T1, wmlpc1, start=True, stop=False)
            nc.tensor.matmul(pt[:BLOC, :H], h0T2, wmlpc2, start=False, stop=True)
            hx_row = work.tile([BLOC, H], f16, tag="hx_row", bufs=1)
            nc.scalar.activation(hx_row, pt[:BLOC, :H], AF.Copy)
            # transpose to hxT chunks
            ptr3 = ps_tr.tile([128, 128], f16, tag="ps_tr")
            nc.tensor.transpose(ptr3[:, :BLOC], hx_row[:, 0:128], ident[:BLOC, :BLOC])
            nc.vector.tensor_copy(hxT[0], ptr3[:, :BLOC])
            ptr4 = ps_tr.tile([128, 128], f16, tag="ps_tr")
            nc.tensor.transpose(ptr4[:72, :BLOC], hx_row[:, 128:200], ident[:BLOC, :BLOC])
            nc.vector.tensor_copy(hxT[1], ptr4[:72, :BLOC])

        def log_softmax_out(S4, out_dram):
            gmax = work.tile([BLOC, 1], f32, tag="gmax")
            nc.vector.tensor_reduce(gmax, S4, axis=AX.X, op=OP.max)
            negm = work.tile([BLOC, 1], f32, tag="negm")
            nc.vector.tensor_scalar_mul(negm, gmax, -1.0)
            e4 = single.tile([BLOC, L], f32, tag="e4")
            sume = work.tile([BLOC, 1], f32, tag="sume")
            nc.scalar.activation(e4, S4, AF.Exp, bias=negm[:, 0:1], accum_out=sume)
            lnz = work.tile([BLOC, 1], f32, tag="lnz")
            nc.scalar.activation(lnz, sume, AF.Ln)
            lse = work.tile([BLOC, 1], f32, tag="lse")
            nc.vector.tensor_tensor(lse, gmax, lnz, OP.add)
            lp4 = single.tile([BLOC, L], f32, tag="e4")
            nc.vector.tensor_scalar(lp4, S4, lse[:, 0:1], None, op0=OP.subtract)
            nc.sync.dma_start(out_dram, lp4)

        # ---------------- the four passes ----------------
        r_row = r_matmul("s", with_hx=False)
        oh, ol = o_rows("s", r_row)
        S4_t0 = work.tile([BLOC, L], f32, tag="S4_tmp", bufs=2)
        score_sweep("s", S4_t0, oh, ol, fill=fill_steps)
        argmax_gather(S4_t0, es16, es16c3)

        r_row = r_matmul("e", with_hx=False)
        oh, ol = o_rows("e", r_row)
        S4_t1 = work.tile([BLOC, L], f32, tag="S4_tmp", bufs=2)
        score_sweep("e", S4_t1, oh, ol)
        argmax_gather(S4_t1, ee16, ee16c3)

        lstm_update()

        r_row = r_matmul("s", with_hx=True)
        oh, ol = o_rows("s", r_row)
        score_sweep("s", S4_a, oh, ol)
        argmax_gather(S4_a, es16, es16c3)

        # issue e2's serial path before lp1's log_softmax so the (vector/
        # scalar) softmax chain doesn't delay the e2 sweep start; lsm then
        # overlaps the e2 sweep
        r_row = r_matmul("e", with_hx=True)
        oh, ol = o_rows("e", r_row)
        log_softmax_out(S4_a, lp1)
        score_sweep("e", S4_b, oh, ol)
        log_softmax_out(S4_b, lp2)

        stage_cm.__exit__(None, None, None)

    nc.compile()
    return nc


def get_program():
    if "nc" not in _cache:
        _cache["nc"] = _build_program()
    return _cache["nc"]


def _split16(x):
    hi = np.asarray(x, np.float32).astype(np.float16)
    lo = (np.asarray(x, np.float32) - hi.astype(np.float32)).astype(np.float16)
    return hi, lo


def make_in_maps(inputs):
    """Per-core input maps: batch shard + trivial host prep (mask, bias splits)."""
    inputs = {k: np.asarray(v) for k, v in inputs.items()}
    enc = np.ascontiguousarray(inputs["encoding_matrix"], dtype=np.float32)
    lens = np.asarray(inputs["passage_lens"]).astype(np.int64)
    pen_full = np.where(np.arange(L)[None, :] < lens[:, None],
                        np.float32(0.0), BIG).astype(np.float32)

    shared = {}
    for tag in ("s", "e"):
        shared[f"w1_{tag}"] = np.ascontiguousarray(inputs[f"w1_{tag}"], np.float32)
        shared[f"b1_{tag}"] = np.ascontiguousarray(inputs[f"b1_{tag}"], np.float32).reshape(1, -1)
        shared[f"w2_{tag}"] = np.ascontiguousarray(inputs[f"w2_{tag}"], np.float32)
        b2h, b2l = _split16(inputs[f"b2_{tag}"])
        shared[f"b2h_{tag}"] = b2h.reshape(1, -1)
        shared[f"b2l_{tag}"] = b2l.reshape(1, -1)
        shared[f"w3_{tag}"] = np.ascontiguousarray(inputs[f"w3_{tag}"], np.float32)
        b3h, b3l = _split16(inputs[f"b3_{tag}"])
        shared[f"b3h_{tag}"] = b3h.reshape(1, -1)
        shared[f"b3l_{tag}"] = b3l.reshape(1, -1)
        shared[f"wd_{tag}"] = np.ascontiguousarray(inputs[f"wd_{tag}"], np.float32)
    shared["w_ih"] = np.ascontiguousarray(inputs["w_ih"], np.float32)
    shared["b_lstm"] = np.ascontiguousarray(inputs["b_lstm"], np.float32).reshape(1, -1)
    shared["w_mlp"] = np.ascontiguousarray(inputs["w_mlp"], np.float32)
    shared["b_mlp"] = np.ascontiguousarray(inputs["b_mlp"], np.float32).reshape(1, -1)

    in_maps = []
    for core in range(NCORES):
        sl = slice(core * BLOC, (core + 1) * BLOC)
        m = dict(shared)
        m["enc"] = np.ascontiguousarray(enc[sl])
        m["pen"] = np.ascontiguousarray(pen_full[sl])
        in_maps.append(m)
    return in_maps


def run_on_hw(inputs, trace=False):
    from concourse import bass_utils
    nc = get_program()
    in_maps = make_in_maps(inputs)
    res = bass_utils.run_bass_kernel_spmd(nc, in_maps, core_ids=list(range(NCORES)),
                                          trace=trace)
    lp1 = np.concatenate([res.results[c]["lp1"] for c in range(NCORES)], axis=0)
    lp2 = np.concatenate([res.results[c]["lp2"] for c in range(NCORES)], axis=0)
    return (np.asarray(lp1, np.float32), np.asarray(lp2, np.float32)), res


def kernel(**inputs):
    out, _ = run_on_hw(inputs, trace=False)
    return out


# revision 41
# speedup vs baseline: 1.0050x; 1.0050x over previous
"""Trainium2 Bass kernel for nn_Decoder_86921548137026.

Dynamic decoder: NITER=2 iterations of (maxout pointer scoring over L=1024
positions -> argmax -> gather -> LSTM cell), followed by log_softmax over the
final start/end scores.

Sharding: data-parallel over batch B=32 across 8 cores (4 batches/core),
weights replicated.

v2 changes vs baseline:
  - serial path (r = tanh(cat@wd), o = r@w1r+b1, LSTM, mlp) flipped to row
    layout [4, n] with fp16 weights: ~10 small fp16 matmuls instead of ~20
    1us fp32 column matmuls (validated offline: rel 3.1e-4 end to end)
  - biases folded as ones-rows in the stationary operand (b1/b_lstm/b_mlp)
  - argmax gather: one dynamic column DMA per (batch, chunkset) dispatched
    round-robin across engine queues instead of 16 serialized sync DMAs
  - S4 row assembly via PE transpose + single DMA per batch (was 8 column
    DMAs per batch)
  - E cast f32->fp16 on vector/scalar engines (was gpsimd, 5x slower), with
    batch-0-first ordering and early sweep start; weight DMAs ordered so the
    first sweep only waits on its own weights
"""

import numpy as np

H = 200
PMX = 8
B = 32
L = 1024
BIG = np.float32(1e30)
NCORES = 8
BLOC = B // NCORES          # 4 batches per core
NLT = L // 128              # 8 l-tiles per batch
# stage-1/2 output channels = H*P = 1600, swept in PSUM-bank-sized chunks
NCH = [(0, 512), (512, 512), (1024, 512), (1536, 64)]
# h-slice of m1/m2 produced by each n-chunk (1600 = 200h * 8p, h-major)
HSL = [(0, 64), (64, 64), (128, 64), (192, 8)]

_cache = {}


def _build_program():
    import contextlib
    import concourse.mybir as mybir
    import concourse.tile as tile
    from concourse import bacc
    from concourse.bass import ds
    from concourse.masks import make_identity

    f32 = mybir.dt.float32
    f16 = mybir.dt.float16
    u32 = mybir.dt.uint32
    AF = mybir.ActivationFunctionType
    OP = mybir.AluOpType
    AX = mybir.AxisListType

    nc = bacc.Bacc("TRN2", target_bir_lowering=False, debug=False,
                   enable_asserts=False, num_devices=NCORES)

    # ---------------- DRAM I/O ----------------
    enc = nc.dram_tensor("enc", [BLOC, 2 * H, L], f32, kind="ExternalInput").ap()
    pen = nc.dram_tensor("pen", [BLOC, L], f32, kind="ExternalInput").ap()
    wt = {}
    for tag in ("s", "e"):
        wt[f"w1_{tag}"] = nc.dram_tensor(f"w1_{tag}", [3 * H, H * PMX], f32, kind="ExternalInput").ap()
        wt[f"b1_{tag}"] = nc.dram_tensor(f"b1_{tag}", [1, H * PMX], f32, kind="ExternalInput").ap()
        wt[f"w2_{tag}"] = nc.dram_tensor(f"w2_{tag}", [H, H * PMX], f32, kind="ExternalInput").ap()
        wt[f"b2h_{tag}"] = nc.dram_tensor(f"b2h_{tag}", [1, H * PMX], f16, kind="ExternalInput").ap()
        wt[f"b2l_{tag}"] = nc.dram_tensor(f"b2l_{tag}", [1, H * PMX], f16, kind="ExternalInput").ap()
        wt[f"w3_{tag}"] = nc.dram_tensor(f"w3_{tag}", [2 * H, PMX], f32, kind="ExternalInput").ap()
        wt[f"b3h_{tag}"] = nc.dram_tensor(f"b3h_{tag}", [1, PMX], f16, kind="ExternalInput").ap()
        wt[f"b3l_{tag}"] = nc.dram_tensor(f"b3l_{tag}", [1, PMX], f16, kind="ExternalInput").ap()
        wt[f"wd_{tag}"] = nc.dram_tensor(f"wd_{tag}", [5 * H, H], f32, kind="ExternalInput").ap()
    wt["w_ih"] = nc.dram_tensor("w_ih", [4 * H, 4 * H], f32, kind="ExternalInput").ap()
    wt["b_lstm"] = nc.dram_tensor("b_lstm", [1, 4 * H], f32, kind="ExternalInput").ap()
    wt["w_mlp"] = nc.dram_tensor("w_mlp", [H, H], f32, kind="ExternalInput").ap()
    wt["b_mlp"] = nc.dram_tensor("b_mlp", [1, H], f32, kind="ExternalInput").ap()

    lp1 = nc.dram_tensor("lp1", [BLOC, L], f32, kind="ExternalOutput").ap()
    lp2 = nc.dram_tensor("lp2", [BLOC, L], f32, kind="ExternalOutput").ap()

    with tile.TileContext(nc) as tc, contextlib.ExitStack() as ctx:
        const = ctx.enter_context(tc.tile_pool(name="const", bufs=1))
        wpool = ctx.enter_context(tc.tile_pool(name="wpool", bufs=1))
        epool = ctx.enter_context(tc.tile_pool(name="epool", bufs=1))
        work = ctx.enter_context(tc.tile_pool(name="work", bufs=3))
        single = ctx.enter_context(tc.tile_pool(name="single", bufs=1))
        keep = ctx.enter_context(tc.tile_pool(name="keep", bufs=1))
        ps_a = ctx.enter_context(tc.tile_pool(name="ps_a", bufs=3, space="PSUM"))
        ps_b = ctx.enter_context(tc.tile_pool(name="ps_b", bufs=2, space="PSUM"))
        ps_tr = ctx.enter_context(tc.tile_pool(name="ps_tr", bufs=2, space="PSUM"))
        ps_d = ctx.enter_context(tc.tile_pool(name="ps_d", bufs=1, space="PSUM"))
        stage_cm = tc.tile_pool(name="stage", bufs=2)
        stage = stage_cm.__enter__()

        # ---------------- constants ----------------
        ident = const.tile([128, 128], f16, name="ident")
        make_identity(nc, ident)
        ident32 = const.tile([128, 128], f32, name="ident32")
        make_identity(nc, ident32)

        # penalty mask in row layout [BLOC, L]
        pen_row = const.tile([BLOC, L], f32, name="pen_row")
        nc.sync.dma_start(pen_row, pen)

        # ---------------- load + cast E (per batch, fp16) ----------------
        # big tile [128, 3, L] per batch (chunks c0..c2) + c3 [18, L] with
        # ones rows 16,17 (o-fold lhsT rows); one batched DMA per batch,
        # all casts on the vector engine (scalar is busy with s-weights)
        E16 = []
        E16c3 = []
        es16 = keep.tile([128, 3, BLOC], f16, name="es16")
        es16c3 = keep.tile([17, BLOC], f16, name="es16c3")
        nc.vector.memset(es16c3, 1.0)              # row 16 = b_lstm fold row
        ee16 = keep.tile([128, 3, BLOC], f16, name="ee16")
        ee16c3 = keep.tile([16, BLOC], f16, name="ee16c3")
        def load_E(b, defer=None):
            est = stage.tile([128, 3, L], f32, tag="estg", bufs=2)
            for c in range(3):
                nc.sync.dma_start(est[:, c, :], enc[b, 128 * c:128 * (c + 1), :])
            et = epool.tile([128, 3, L], f16, name=f"E16_{b}")
            if defer is None:
                nc.vector.tensor_copy(et, est)
            else:
                # cast deferred into sweep-s1 fill steps on the scalar queue
                # (keeps the pre-sweep vector queue clear for the reduces)
                defer.append(lambda et=et, est=est:
                             nc.scalar.activation(et, est, AF.Copy))
            E16.append(et)
            # initial es = E[:, 0, :] straight from the f32 stage (doesn't
            # wait on the big cast)
            nc.vector.tensor_copy(es16[:, :, b:b + 1], est[:, :, 0:1])
            est2 = stage.tile([128, 1600], f32, tag="stg")
            dma_eng2 = nc.scalar if b % 2 else nc.sync
            dma_eng2.dma_start(est2[:16, :L], enc[b, 384:400, :])
            ec3 = epool.tile([18, L], f16, name=f"E16c3_{b}")
            nc.vector.memset(ec3, 1.0)
            if defer is None:
                nc.vector.tensor_copy(ec3[:16], est2[:16, :L])
            else:
                defer.append(lambda ec3=ec3, est2=est2:
                             nc.scalar.activation(ec3[:16], est2[:16, :L], AF.Copy))
            nc.vector.tensor_copy(es16c3[:16, b:b + 1], est2[:16, 0:1])
            E16c3.append(ec3)

        # batches 0,1 first; 2,3 after the s-weights so the first sweep's
        # weight DMAs aren't starved behind all 6.5MB of E traffic
        load_E(0)
        load_E(1)

        # ---------------- weights ----------------
        # order: s-scoring weights first (first sweep waits only on these).
        # tag "s" casts: scalar (sweep weights) + vector (serial weights);
        # tag "e" + lstm casts: gpsimd (idle during sweep s1, done by gap 1)
        W = {}

        def cast(eng, dst, src):
            if eng is nc.scalar:
                nc.scalar.activation(dst, src, AF.Copy)
            else:
                eng.tensor_copy(dst, src)

        def load_scoring(tag, dma, csw, cse, stg):
            # generator: yields after each dma+cast step so the caller can
            # interleave emission with sweep instructions
            # stage-1 rhs rows 0..383 as fp16 [128, 3, 1600]
            w1m = wpool.tile([128, 3, H * PMX], f16, name=f"w1m_{tag}")
            for c in range(3):
                wst = stage.tile([128, 1600], f32, tag=stg, bufs=2)
                dma.dma_start(wst, wt[f"w1_{tag}"][128 * c:128 * (c + 1)])
                cast(csw, w1m[:, c, :], wst)
                yield
            W[f"w1m_{tag}"] = w1m
            # shared c3 rhs: rows 0..15 = W1[384:400] fp16, rows 16/17 = o hi/lo
            # (double-buffered by batch parity to relax WAR stalls)
            wst = stage.tile([128, 1600], f32, tag=stg, bufs=2)
            dma.dma_start(wst[:16], wt[f"w1_{tag}"][384:400])
            c3pair = []
            for pi in range(2):
                c3 = wpool.tile([18, H * PMX], f16, name=f"c3_{tag}_{pi}")
                cast(csw, c3[:16], wst[:16])
                c3pair.append(c3)
            yield
            W[f"c3_{tag}"] = c3pair
            # o-matmul rhs (fp16): rows 400..599 of w1, + b1 as ones-row 72 of c2
            wst = stage.tile([128, 1600], f32, tag=stg, bufs=2)
            dma.dma_start(wst, wt[f"w1_{tag}"][400:528])
            w1rc1 = wpool.tile([128, H * PMX], f16, name=f"w1r16c1_{tag}")
            cast(cse, w1rc1, wst)
            yield
            wst = stage.tile([128, 1600], f32, tag=stg, bufs=2)
            dma.dma_start(wst[:72], wt[f"w1_{tag}"][528:600])
            dma.dma_start(wst[72:73], wt[f"b1_{tag}"])
            w1rc2 = wpool.tile([73, H * PMX], f16, name=f"w1r16c2_{tag}")
            cast(cse, w1rc2, wst[:73])
            yield
            W[f"w1r_{tag}"] = (w1rc1, w1rc2)
            # wd fp16 k-chunk tiles over cat(hx[0:200], es[200:600], ee[600:1000])
            wd_chunks = []
            for (k0, kn) in [(0, 128), (128, 72), (200, 128), (328, 128), (456, 128), (584, 16),
                             (600, 128), (728, 128), (856, 128), (984, 16)]:
                wst = stage.tile([128, 1600], f32, tag=stg, bufs=2)
                dma.dma_start(wst[:kn, :H], wt[f"wd_{tag}"][k0:k0 + kn])
                t = wpool.tile([kn, H], f16, name=f"wd16_{tag}_{k0}")
                cast(cse, t, wst[:kn, :H])
                wd_chunks.append((k0, kn, t))
                yield
            W[f"wd_{tag}"] = wd_chunks

        def load_scoring2(tag, dma, csw, stg):
            # stage-2 rhs
            w2c1 = wpool.tile([128, H * PMX], f16, name=f"w2c1_{tag}")
            wst = stage.tile([128, 1600], f32, tag=stg, bufs=2)
            dma.dma_start(wst, wt[f"w2_{tag}"][:128])
            cast(csw, w2c1, wst)
            yield
            W[f"w2c1_{tag}"] = w2c1
            w2c2 = wpool.tile([74, H * PMX], f16, name=f"w2c2_{tag}")
            wst = stage.tile([128, 1600], f32, tag=stg, bufs=2)
            dma.dma_start(wst[:72], wt[f"w2_{tag}"][128:200])
            cast(csw, w2c2[:72], wst[:72])
            dma.dma_start(w2c2[72:73, :], wt[f"b2h_{tag}"])
            dma.dma_start(w2c2[73:74, :], wt[f"b2l_{tag}"])
            yield
            W[f"w2c2_{tag}"] = w2c2
            # stage-3 rhs chunks
            wst = stage.tile([128, 1600], f32, tag=stg, bufs=2)
            dma.dma_start(wst[:, 0:8], wt[f"w3_{tag}"][0:128])
            dma.dma_start(wst[:72, 8:16], wt[f"w3_{tag}"][128:200])
            dma.dma_start(wst[:, 16:24], wt[f"w3_{tag}"][200:328])
            dma.dma_start(wst[:72, 24:32], wt[f"w3_{tag}"][328:400])
            w3c1 = wpool.tile([128, PMX], f16, name=f"w3c1_{tag}")
            cast(csw, w3c1, wst[:, 0:8])
            w3c2 = wpool.tile([74, PMX], f16, name=f"w3c2_{tag}")
            cast(csw, w3c2[:72], wst[:72, 8:16])
            yield
            dma.dma_start(w3c2[72:73, :], wt[f"b3h_{tag}"])
            dma.dma_start(w3c2[73:74, :], wt[f"b3l_{tag}"])
            w3c3 = wpool.tile([128, PMX], f16, name=f"w3c3_{tag}")
            cast(csw, w3c3, wst[:, 16:24])
            w3c4 = wpool.tile([72, PMX], f16, name=f"w3c4_{tag}")
            cast(csw, w3c4, wst[:72, 24:32])
            yield
            W[f"w3_{tag}"] = (w3c1, w3c2, w3c3, w3c4)

        def load_lstm(dma, cse):
            # LSTM weights as fp16 rhs [kn, 800] over rows = cat(es, ee);
            # es-c3 chunk carries b_lstm as ones-row 16
            ih_chunks = []
            for (k0, kn) in [(0, 128), (128, 128), (256, 128), (384, 16),
                             (400, 128), (528, 128), (656, 128), (784, 16)]:
                wst = stage.tile([128, 1600], f32, tag="wstg_e", bufs=2)
                dma.dma_start(wst[:kn, :800], wt["w_ih"][k0:k0 + kn])
                rows = kn + 1 if k0 == 384 else kn
                t = wpool.tile([rows, 4 * H], f16, name=f"wih16_{k0}")
                if k0 == 384:
                    dma.dma_start(wst[16:17, :800], wt["b_lstm"])
                cast(cse, t[:rows], wst[:rows, :800])
                ih_chunks.append((k0, kn, t))
                yield
            # mlp fp16 rhs [kn, 200]; c2 carries b_mlp as ones-row 72
            wst = stage.tile([128, 1600], f32, tag="wstg_e", bufs=2)
            dma.dma_start(wst[:, :H], wt["w_mlp"][0:128])
            wmlpc1 = wpool.tile([128, H], f16, name="wmlp16c1")
            cast(cse, wmlpc1, wst[:, :H])
            yield
            wst = stage.tile([128, 1600], f32, tag="wstg_e", bufs=2)
            dma.dma_start(wst[:72, :H], wt["w_mlp"][128:200])
            dma.dma_start(wst[72:73, :H], wt["b_mlp"])
            wmlpc2 = wpool.tile([73, H], f16, name="wmlp16c2")
            cast(cse, wmlpc2, wst[:73, :H])
            W["ih_chunks"] = ih_chunks
            W["wmlp"] = (wmlpc1, wmlpc2)
            yield

        # s-tag weights emitted eagerly (first sweep waits on them); e-tag +
        # lstm weights emitted as fill steps interleaved into sweep s1
        import itertools
        for _ in load_scoring("s", nc.sync, nc.scalar, nc.vector, "stg"):
            pass
        for _ in load_scoring2("s", nc.sync, nc.scalar, "stg"):
            pass
        edefer = []
        load_E(2, edefer)
        load_E(3, edefer)

        def edefer_gen():
            for fn in edefer:
                fn()
                yield
        fill_steps = itertools.chain(
            edefer_gen(),
            load_scoring("e", nc.sync, nc.scalar, nc.scalar, "wstg_e"),
            load_scoring2("e", nc.sync, nc.scalar, "wstg_e"),
            load_lstm(nc.sync, nc.scalar))

        # ---------------- persistent state tiles ----------------
        hxT = [keep.tile([128, BLOC], f16, name="hxT16_0"),
               keep.tile([72, BLOC], f16, name="hxT16_1")]
        rT1 = keep.tile([128, BLOC], f16, name="rT1")
        rT2 = keep.tile([73, BLOC], f16, name="rT2")
        nc.gpsimd.memset(rT2, 1.0)                 # row 72 = b1 fold row
        h0T1 = keep.tile([128, BLOC], f16, name="h0T1")
        h0T2 = keep.tile([73, BLOC], f16, name="h0T2")
        nc.gpsimd.memset(h0T2, 1.0)                # row 72 = b_mlp fold row

        # es/ee init happened inline in the E load loop (from the f32 stage)
        nc.vector.tensor_copy(ee16, es16)
        nc.vector.tensor_copy(ee16c3, es16c3[:16])

        # manually rotated work slots; m1c2 slots carry persistent ones rows
        # 72/73 (paired with the b2/b3 hi+lo rhs rows)
        m1_slots, m2_slots, m1c2_slots = [], [], []
        for i in range(4):
            m1_slots.append(keep.tile([128, H], f16, name=f"m1_slot{i}"))
            m2_slots.append(keep.tile([128, H], f16, name=f"m2_slot{i}"))
            t = keep.tile([74, 128], f16, name=f"m1c2_slot{i}")
            nc.gpsimd.memset(t, 1.0)
            m1c2_slots.append(t)
        S4_a = keep.tile([BLOC, L], f32, name="S4_a")
        S4_b = keep.tile([BLOC, L], f32, name="S4_b")

        dma_engines = [nc.sync, nc.scalar, nc.gpsimd]

        # ---------------- helpers ----------------
        def cat_chunks(tag, with_hx):
            """(lhsT [kn,4] fp16, wd16 [kn,H]) pairs for r = tanh(cat @ wd)."""
            ops = []
            for (k0, kn, wtile) in W[f"wd_{tag}"]:
                if k0 < 200:
                    if not with_hx:
                        continue
                    lhsT = hxT[0] if k0 == 0 else hxT[1]
                elif k0 < 600:
                    c = (k0 - 200) // 128
                    lhsT = es16[:, c, :] if c < 3 else es16c3[:16]
                else:
                    c = (k0 - 600) // 128
                    lhsT = ee16[:, c, :] if c < 3 else ee16c3
                ops.append((lhsT, wtile))
            return ops

        def r_matmul(tag, with_hx):
            """r_row = tanh(cat @ wd) -> [4, H] fp16 sbuf."""
            ops = cat_chunks(tag, with_hx)
            pt = ps_d.tile([128, 512], f32, tag="ps_ser")
            for i, (lhsT, rhs) in enumerate(ops):
                nc.tensor.matmul(pt[:BLOC, :H], lhsT, rhs,
                                 start=(i == 0), stop=(i == len(ops) - 1))
            r_row = work.tile([BLOC, H], f16, tag="r_row", bufs=1)
            nc.scalar.activation(r_row, pt[:BLOC, :H], AF.Tanh)
            return r_row

        def o_rows(tag, r_row):
            """o = r@w1r + b1 (f32 psum) -> fp16 hi/lo rows [BLOC, 1600]."""
            # transpose r to column chunks [128,4], [72,4] (+ones row 72)
            ptr = ps_tr.tile([128, 128], f16, tag="ps_tr")
            nc.tensor.transpose(ptr[:, :BLOC], r_row[:, 0:128], ident[:BLOC, :BLOC])
            nc.vector.tensor_copy(rT1, ptr[:, :BLOC])
            ptr2 = ps_tr.tile([128, 128], f16, tag="ps_tr")
            nc.tensor.transpose(ptr2[:72, :BLOC], r_row[:, 128:200], ident[:BLOC, :BLOC])
            nc.vector.tensor_copy(rT2[:72], ptr2[:72, :BLOC])
            w1rc1, w1rc2 = W[f"w1r_{tag}"]
            oh = single.tile([BLOC, H * PMX], f16, tag="oh")
            ol = single.tile([BLOC, H * PMX], f16, tag="ol")
            for (n0, nn) in NCH:
                pt = ps_d.tile([128, 512], f32, tag="ps_ser")
                nc.tensor.matmul(pt[:BLOC, :nn], rT1, w1rc1[:, n0:n0 + nn], start=True, stop=False)
                nc.tensor.matmul(pt[:BLOC, :nn], rT2, w1rc2[:, n0:n0 + nn], start=False, stop=True)
                nc.scalar.activation(oh[:, n0:n0 + nn], pt[:BLOC, :nn], AF.Copy)
                nc.vector.tensor_tensor(ol[:, n0:n0 + nn], pt[:BLOC, :nn], oh[:, n0:n0 + nn], OP.subtract)
            return oh, ol

        def score_sweep(tag, S4, oh, ol, fill=None):
            """Maxout scoring, software-pipelined across (b, lt) tiles."""
            w1m = W[f"w1m_{tag}"]
            c3pair = W[f"c3_{tag}"]
            w2c1 = W[f"w2c1_{tag}"]
            w2c2 = W[f"w2c2_{tag}"]
            w3c1, w3c2, w3c3, w3c4 = W[f"w3_{tag}"]
            NT = BLOC * NLT
            st = [dict() for _ in range(NT)]
            strips = {}

            def g1(i):
                b, lt = divmod(i, NLT)
                c3rhs = c3pair[b % 2]
                if lt == 0:
                    nc.sync.dma_start(c3rhs[16:17, :], oh[b:b + 1, :])
                    nc.sync.dma_start(c3rhs[17:18, :], ol[b:b + 1, :])
                lsl = slice(128 * lt, 128 * (lt + 1))
                m1 = m1_slots[i % 4]
                for ni, (n0, nn) in enumerate(NCH):
                    pa = ps_a.tile([128, 512], f32, tag="ps_s1")
                    for c in range(3):
                        nc.tensor.matmul(pa[:, :nn], E16[b][:, c, lsl], w1m[:, c, n0:n0 + nn],
                                         start=(c == 0), stop=False)
                    nc.tensor.matmul(pa[:, :nn], E16c3[b][:, lsl], c3rhs[:, n0:n0 + nn],
                                     start=False, stop=True)
                    h0, hn = HSL[ni]
                    nc.vector.tensor_reduce(
                        m1[:, h0:h0 + hn],
                        pa[:, :nn].rearrange("p (h q) -> p h q", q=PMX),
                        axis=AX.X, op=OP.max)
                st[i]["m1"] = m1

            def g2(i):
                m1 = st[i]["m1"]
                pt1 = ps_tr.tile([128, 128], f16, tag="ps_tr")
                nc.tensor.transpose(pt1, m1[:, 0:128], ident)
                m1c1 = work.tile([128, 128], f16, tag="m1c1")
                nc.scalar.activation(m1c1, pt1, AF.Copy)
                pt2 = ps_tr.tile([128, 128], f16, tag="ps_tr")
                nc.tensor.transpose(pt2[:72], m1[:, 128:200], ident)
                m1c2 = m1c2_slots[i % 4]
                nc.scalar.activation(m1c2[:72], pt2[:72], AF.Copy)
                m2 = m2_slots[i % 4]
                for ni, (n0, nn) in enumerate(NCH):
                    pb = ps_b.tile([128, 512], f32, tag="ps_s2")
                    nc.tensor.matmul(pb[:, :nn], m1c1, w2c1[:, n0:n0 + nn], start=True, stop=False)
                    nc.tensor.matmul(pb[:, :nn], m1c2, w2c2[:, n0:n0 + nn], start=False, stop=True)
                    h0, hn = HSL[ni]
                    nc.vector.tensor_reduce(
                        m2[:, h0:h0 + hn],
                        pb[:, :nn].rearrange("p (h q) -> p h q", q=PMX),
                        axis=AX.X, op=OP.max)
                st[i]["m1c1"] = m1c1
                st[i]["m1c2"] = m1c2
                st[i]["m2"] = m2

            def g3(i):
                b, lt = divmod(i, NLT)
                m2 = st[i]["m2"]
                pt3 = ps_tr.tile([128, 128], f16, tag="ps_tr")
                nc.tensor.transpose(pt3, m2[:, 0:128], ident)
                m2c1 = work.tile([128, 128], f16, tag="m2c1")
                nc.scalar.activation(m2c1, pt3, AF.Copy)
                pt4 = ps_tr.tile([128, 128], f16, tag="ps_tr")
                nc.tensor.transpose(pt4[:72], m2[:, 128:200], ident)
                m2c2 = work.tile([72, 128], f16, tag="m2c2")
                nc.scalar.activation(m2c2, pt4[:72], AF.Copy)
                if lt == 0:
                    strips[b] = ps_d.tile([128, 8 * NLT], f32, tag="ps_ser", name="s3strip")
                psl = strips[b][:, 8 * lt:8 * (lt + 1)]
                nc.tensor.matmul(psl, st[i]["m1c1"], w3c1, start=True, stop=False)
                nc.tensor.matmul(psl, st[i]["m1c2"], w3c2, start=False, stop=False)
                nc.tensor.matmul(psl, m2c1, w3c3, start=False, stop=False)
                nc.tensor.matmul(psl, m2c2, w3c4, start=False, stop=True)
                st[i].clear()
                if lt == NLT - 1:
                    Sb = work.tile([128, NLT], f32, tag="Sb")
                    nc.vector.tensor_reduce(Sb,
                                            strips[b].rearrange("p (t q) -> p t q", q=PMX),
                                            axis=AX.X, op=OP.max)
                    ptb = ps_tr.tile([NLT, 128], f32, tag="ps_tr")
                    nc.tensor.transpose(ptb, Sb, ident32)
                    s4stg = work.tile([NLT, 128], f32, tag="s4stg")
                    nc.scalar.activation(s4stg, ptb, AF.Copy)
                    dma_engines[b % 3].dma_start(S4[b:b + 1, :], s4stg)
            for i in range(NT + 2):
                if i < NT:
                    g1(i)
                if 1 <= i < NT + 1:
                    g2(i - 1)
                if 2 <= i:
                    g3(i - 2)
                if fill is not None:
                    next(fill, None)
            if fill is not None:
                for _ in fill:
                    pass
            nc.vector.tensor_tensor(S4, S4, pen_row, OP.subtract)

        def argmax_gather(S4, dstbig, dstc3):
            """argmax over S4 rows; gather E columns (fp16)."""
            mx8 = work.tile([BLOC, 8], f32, tag="mx8")
            idx8 = work.tile([BLOC, 8], u32, tag="idx8")
            nc.vector.max(out=mx8, in_=S4)
            nc.vector.max_index(out=idx8, in_max=mx8, in_values=S4)
            for b in range(BLOC):
                reg = nc.values_load(idx8[b:b + 1, 0:1], min_val=0, max_val=L - 1,
                                     skip_runtime_bounds_check=True)
                dma_engines[(2 * b) % 3].dma_start(
                    dstbig[:, :, b:b + 1], E16[b][:, :, ds(reg, 1)])
                dma_engines[(2 * b + 1) % 3].dma_start(
                    dstc3[:16, b:b + 1], E16c3[b][:16, ds(reg, 1)])

        def lstm_update():
            """hx via LSTM cell with hx0=cx0=0 (f-gate and w_hh drop out)."""
            # gates row-layout: psum [4, 200] for i; [4, 400] for g,o
            pt_i = ps_d.tile([128, 512], f32, tag="ps_ser")
            pt_go = ps_d.tile([128, 512], f32, tag="ps_ser")
            lhs_for = []
            for (k0, kn, wtile) in W["ih_chunks"]:
                if k0 < 400:
                    c = k0 // 128
                    lhsT = es16[:, c, :] if c < 3 else es16c3  # [17,4] w/ ones
                else:
                    c = (k0 - 400) // 128
                    lhsT = ee16[:, c, :] if c < 3 else ee16c3
                lhs_for.append((lhsT, wtile, kn + (1 if k0 == 384 else 0)))
            n = len(lhs_for)
            for i, (lhsT, wtile, rows) in enumerate(lhs_for):
                nc.tensor.matmul(pt_i[:BLOC, :H], lhsT, wtile[:rows, 0:H],
                                 start=(i == 0), stop=(i == n - 1))
            for i, (lhsT, wtile, rows) in enumerate(lhs_for):
                nc.tensor.matmul(pt_go[:BLOC, :2 * H], lhsT, wtile[:rows, 2 * H:4 * H],
                                 start=(i == 0), stop=(i == n - 1))
            ig = work.tile([BLOC, H], f32, tag="ig", bufs=1)
            nc.scalar.activation(ig, pt_i[:BLOC, :H], AF.Sigmoid)
            gg = work.tile([BLOC, H], f32, tag="gg", bufs=1)
            nc.scalar.activation(gg, pt_go[:BLOC, 0:H], AF.Tanh)
            og = work.tile([BLOC, H], f32, tag="og", bufs=1)
            nc.scalar.activation(og, pt_go[:BLOC, H:2 * H], AF.Sigmoid)
            cx = work.tile([BLOC, H], f32, tag="cx", bufs=1)
            nc.vector.tensor_tensor(cx, ig, gg, OP.mult)
            tcx = work.tile([BLOC, H], f32, tag="tcx", bufs=1)
            nc.scalar.activation(tcx, cx, AF.Tanh)
            h0 = work.tile([BLOC, H], f16, tag="h0", bufs=1)
            nc.vector.tensor_tensor(h0, og, tcx, OP.mult)
            # transpose h0 -> column chunks (+ones row 72 for b_mlp)
            ptr = ps_tr.tile([128, 128], f16, tag="ps_tr")
            nc.tensor.transpose(ptr[:, :BLOC], h0[:, 0:128], ident[:BLOC, :BLOC])
            nc.vector.tensor_copy(h0T1, ptr[:, :BLOC])
            ptr2 = ps_tr.tile([128, 128], f16, tag="ps_tr")
            nc.tensor.transpose(ptr2[:72, :BLOC], h0[:, 128:200], ident[:BLOC, :BLOC])
            nc.vector.tensor_copy(h0T2[:72], ptr2[:72, :BLOC])
            # mlp: hx = h0 @ w_mlp + b_mlp
            pt = ps_d.tile([128, 512], f32, tag="ps_ser")
            wmlpc1, wmlpc2 = W["wmlp"]
            nc.tensor.matmul(pt[:BLOC, :H], h0T1, wmlpc1, start=True, stop=False)
            nc.tensor.matmul(pt[:BLOC, :H], h0T2, wmlpc2, start=False, stop=True)
            hx_row = work.tile([BLOC, H], f16, tag="hx_row", bufs=1)
            nc.scalar.activation(hx_row, pt[:BLOC, :H], AF.Copy)
            # transpose to hxT chunks
            ptr3 = ps_tr.tile([128, 128], f16, tag="ps_tr")
            nc.tensor.transpose(ptr3[:, :BLOC], hx_row[:, 0:128], ident[:BLOC, :BLOC])
            nc.vector.tensor_copy(hxT[0], ptr3[:, :BLOC])
            ptr4 = ps_tr.tile([128, 128], f16, tag="ps_tr")
            nc.tensor.transpose(ptr4[:72, :BLOC], hx_row[:, 128:200], ident[:BLOC, :BLOC])
            nc.vector.tensor_copy(hxT[1], ptr4[:72, :BLOC])

        def log_softmax_out(S4, out_dram):
            gmax = work.tile([BLOC, 1], f32, tag="gmax")
            nc.vector.tensor_reduce(gmax, S4, axis=AX.X, op=OP.max)
            negm = work.tile([BLOC, 1], f32, tag="negm")
            nc.vector.tensor_scalar_mul(negm, gmax, -1.0)
            e4 = single.tile([BLOC, L], f32, tag="e4")
            sume = work.tile([BLOC, 1], f32, tag="sume")
            nc.scalar.activation(e4, S4, AF.Exp, bias=negm[:, 0:1], accum_out=sume)
            lnz = work.tile([BLOC, 1], f32, tag="lnz")
            nc.scalar.activation(lnz, sume, AF.Ln)
            lse = work.tile([BLOC, 1], f32, tag="lse")
            nc.vector.tensor_tensor(lse, gmax, lnz, OP.add)
            lp4 = single.tile([BLOC, L], f32, tag="e4")
            nc.vector.tensor_scalar(lp4, S4, lse[:, 0:1], None, op0=OP.subtract)
            nc.sync.dma_start(out_dram, lp4)

        # ---------------- the four passes ----------------
        r_row = r_matmul("s", with_hx=False)
        oh, ol = o_rows("s", r_row)
        S4_t0 = work.tile([BLOC, L], f32, tag="S4_tmp", bufs=2)
        score_sweep("s", S4_t0, oh, ol, fill=fill_steps)
        argmax_gather(S4_t0, es16, es16c3)

        r_row = r_matmul("e", with_hx=False)
        oh, ol = o_rows("e", r_row)
        S4_t1 = work.tile([BLOC, L], f32, tag="S4_tmp", bufs=2)
        score_sweep("e", S4_t1, oh, ol)
        argmax_gather(S4_t1, ee16, ee16c3)

        lstm_update()

        r_row = r_matmul("s", with_hx=True)
        oh, ol = o_rows("s", r_row)
        score_sweep("s", S4_a, oh, ol)
        argmax_gather(S4_a, es16, es16c3)

        # issue e2's serial path before lp1's log_softmax so the (vector/
        # scalar) softmax chain doesn't delay the e2 sweep start; lsm then
        # overlaps the e2 sweep
        r_row = r_matmul("e", with_hx=True)
        oh, ol = o_rows("e", r_row)
        log_softmax_out(S4_a, lp1)
        score_sweep("e", S4_b, oh, ol)
        log_softmax_out(S4_b, lp2)

        stage_cm.__exit__(None, None, None)

    nc.compile()
    return nc


def get_program():
    if "nc" not in _cache:
        _cache["nc"] = _build_program()
    return _cache["nc"]


def _split16(x):
    hi = np.asarray(x, np.float32).astype(np.float16)
    lo = (np.asarray(x, np.float32) - hi.astype(np.float32)).astype(np.float16)
    return hi, lo


def make_in_maps(inputs):
    """Per-core input maps: batch shard + trivial host prep (mask, bias splits)."""
    inputs = {k: np.asarray(v) for k, v in inputs.items()}
    enc = np.ascontiguousarray(inputs["encoding_matrix"], dtype=np.float32)
    lens = np.asarray(inputs["passage_lens"]).astype(np.int64)
    pen_full = np.where(np.arange(L)[None, :] < lens[:, None],
                        np.float32(0.0), BIG).astype(np.float32)

    shared = {}
    for tag in ("s", "e"):
        shared[f"w1_{tag}"] = np.ascontiguousarray(inputs[f"w1_{tag}"], np.float32)
        shared[f"b1_{tag}"] = np.ascontiguousarray(inputs[f"b1_{tag}"], np.float32).reshape(1, -1)
        shared[f"w2_{tag}"] = np.ascontiguousarray(inputs[f"w2_{tag}"], np.float32)
        b2h, b2l = _split16(inputs[f"b2_{tag}"])
        shared[f"b2h_{tag}"] = b2h.reshape(1, -1)
        shared[f"b2l_{tag}"] = b2l.reshape(1, -1)
        shared[f"w3_{tag}"] = np.ascontiguousarray(inputs[f"w3_{tag}"], np.float32)
        b3h, b3l = _split16(inputs[f"b3_{tag}"])
        shared[f"b3h_{tag}"] = b3h.reshape(1, -1)
        shared[f"b3l_{tag}"] = b3l.reshape(1, -1)
        shared[f"wd_{tag}"] = np.ascontiguousarray(inputs[f"wd_{tag}"], np.float32)
    shared["w_ih"] = np.ascontiguousarray(inputs["w_ih"], np.float32)
    shared["b_lstm"] = np.ascontiguousarray(inputs["b_lstm"], np.float32).reshape(1, -1)
    shared["w_mlp"] = np.ascontiguousarray(inputs["w_mlp"], np.float32)
    shared["b_mlp"] = np.ascontiguousarray(inputs["b_mlp"], np.float32).reshape(1, -1)

    in_maps = []
    for core in range(NCORES):
        sl = slice(core * BLOC, (core + 1) * BLOC)
        m = dict(shared)
        m["enc"] = np.ascontiguousarray(enc[sl])
        m["pen"] = np.ascontiguousarray(pen_full[sl])
        in_maps.append(m)
    return in_maps


def run_on_hw(inputs, trace=False):
    from concourse import bass_utils
    nc = get_program()
    in_maps = make_in_maps(inputs)
    res = bass_utils.run_bass_kernel_spmd(nc, in_maps, core_ids=list(range(NCORES)),
                                          trace=trace)
    lp1 = np.concatenate([res.results[c]["lp1"] for c in range(NCORES)], axis=0)
    lp2 = np.concatenate([res.results[c]["lp2"] for c in range(NCORES)], axis=0)
    return (np.asarray(lp1, np.float32), np.asarray(lp2, np.float32)), res


def kernel(**inputs):
    out, _ = run_on_hw(inputs, trace=False)
    return out


# revision 42
# speedup vs baseline: 1.0050x; 1.0001x over previous
"""Trainium2 Bass kernel for nn_Decoder_86921548137026.

Dynamic decoder: NITER=2 iterations of (maxout pointer scoring over L=1024
positions -> argmax -> gather -> LSTM cell), followed by log_softmax over the
final start/end scores.

Sharding: data-parallel over batch B=32 across 8 cores (4 batches/core),
weights replicated.

v2 changes vs baseline:
  - serial path (r = tanh(cat@wd), o = r@w1r+b1, LSTM, mlp) flipped to row
    layout [4, n] with fp16 weights: ~10 small fp16 matmuls instead of ~20
    1us fp32 column matmuls (validated offline: rel 3.1e-4 end to end)
  - biases folded as ones-rows in the stationary operand (b1/b_lstm/b_mlp)
  - argmax gather: one dynamic column DMA per (batch, chunkset) dispatched
    round-robin across engine queues instead of 16 serialized sync DMAs
  - S4 row assembly via PE transpose + single DMA per batch (was 8 column
    DMAs per batch)
  - E cast f32->fp16 on vector/scalar engines (was gpsimd, 5x slower), with
    batch-0-first ordering and early sweep start; weight DMAs ordered so the
    first sweep only waits on its own weights
"""

import numpy as np

H = 200
PMX = 8
B = 32
L = 1024
BIG = np.float32(1e30)
NCORES = 8
BLOC = B // NCORES          # 4 batches per core
NLT = L // 128              # 8 l-tiles per batch
# stage-1/2 output channels = H*P = 1600, swept in PSUM-bank-sized chunks
NCH = [(0, 512), (512, 512), (1024, 512), (1536, 64)]
# h-slice of m1/m2 produced by each n-chunk (1600 = 200h * 8p, h-major)
HSL = [(0, 64), (64, 64), (128, 64), (192, 8)]

_cache = {}


def _build_program():
    import contextlib
    import concourse.mybir as mybir
    import concourse.tile as tile
    from concourse import bacc
    from concourse.bass import ds
    from concourse.masks import make_identity

    f32 = mybir.dt.float32
    f16 = mybir.dt.float16
    u32 = mybir.dt.uint32
    AF = mybir.ActivationFunctionType
    OP = mybir.AluOpType
    AX = mybir.AxisListType

    nc = bacc.Bacc("TRN2", target_bir_lowering=False, debug=False,
                   enable_asserts=False, num_devices=NCORES)

    # ---------------- DRAM I/O ----------------
    enc = nc.dram_tensor("enc", [BLOC, 2 * H, L], f32, kind="ExternalInput").ap()
    pen = nc.dram_tensor("pen", [BLOC, L], f32, kind="ExternalInput").ap()
    wt = {}
    for tag in ("s", "e"):
        wt[f"w1_{tag}"] = nc.dram_tensor(f"w1_{tag}", [3 * H, H * PMX], f32, kind="ExternalInput").ap()
        wt[f"b1_{tag}"] = nc.dram_tensor(f"b1_{tag}", [1, H * PMX], f32, kind="ExternalInput").ap()
        wt[f"w2_{tag}"] = nc.dram_tensor(f"w2_{tag}", [H, H * PMX], f32, kind="ExternalInput").ap()
        wt[f"b2h_{tag}"] = nc.dram_tensor(f"b2h_{tag}", [1, H * PMX], f16, kind="ExternalInput").ap()
        wt[f"b2l_{tag}"] = nc.dram_tensor(f"b2l_{tag}", [1, H * PMX], f16, kind="ExternalInput").ap()
        wt[f"w3_{tag}"] = nc.dram_tensor(f"w3_{tag}", [2 * H, PMX], f32, kind="ExternalInput").ap()
        wt[f"b3h_{tag}"] = nc.dram_tensor(f"b3h_{tag}", [1, PMX], f16, kind="ExternalInput").ap()
        wt[f"b3l_{tag}"] = nc.dram_tensor(f"b3l_{tag}", [1, PMX], f16, kind="ExternalInput").ap()
        wt[f"wd_{tag}"] = nc.dram_tensor(f"wd_{tag}", [5 * H, H], f32, kind="ExternalInput").ap()
    wt["w_ih"] = nc.dram_tensor("w_ih", [4 * H, 4 * H], f32, kind="ExternalInput").ap()
    wt["b_lstm"] = nc.dram_tensor("b_lstm", [1, 4 * H], f32, kind="ExternalInput").ap()
    wt["w_mlp"] = nc.dram_tensor("w_mlp", [H, H], f32, kind="ExternalInput").ap()
    wt["b_mlp"] = nc.dram_tensor("b_mlp", [1, H], f32, kind="ExternalInput").ap()

    lp1 = nc.dram_tensor("lp1", [BLOC, L], f32, kind="ExternalOutput").ap()
    lp2 = nc.dram_tensor("lp2", [BLOC, L], f32, kind="ExternalOutput").ap()

    with tile.TileContext(nc) as tc, contextlib.ExitStack() as ctx:
        const = ctx.enter_context(tc.tile_pool(name="const", bufs=1))
        wpool = ctx.enter_context(tc.tile_pool(name="wpool", bufs=1))
        epool = ctx.enter_context(tc.tile_pool(name="epool", bufs=1))
        work = ctx.enter_context(tc.tile_pool(name="work", bufs=3))
        single = ctx.enter_context(tc.tile_pool(name="single", bufs=1))
        keep = ctx.enter_context(tc.tile_pool(name="keep", bufs=1))
        ps_a = ctx.enter_context(tc.tile_pool(name="ps_a", bufs=3, space="PSUM"))
        ps_b = ctx.enter_context(tc.tile_pool(name="ps_b", bufs=2, space="PSUM"))
        ps_tr = ctx.enter_context(tc.tile_pool(name="ps_tr", bufs=2, space="PSUM"))
        ps_d = ctx.enter_context(tc.tile_pool(name="ps_d", bufs=1, space="PSUM"))
        stage_cm = tc.tile_pool(name="stage", bufs=2)
        stage = stage_cm.__enter__()

        # ---------------- constants ----------------
        ident = const.tile([128, 128], f16, name="ident")
        make_identity(nc, ident)
        ident32 = const.tile([128, 128], f32, name="ident32")
        make_identity(nc, ident32)

        # penalty mask in row layout [BLOC, L]
        pen_row = const.tile([BLOC, L], f32, name="pen_row")
        nc.sync.dma_start(pen_row, pen)

        # ---------------- load + cast E (per batch, fp16) ----------------
        # big tile [128, 3, L] per batch (chunks c0..c2) + c3 [18, L] with
        # ones rows 16,17 (o-fold lhsT rows); one batched DMA per batch,
        # all casts on the vector engine (scalar is busy with s-weights)
        E16 = []
        E16c3 = []
        es16 = keep.tile([128, 3, BLOC], f16, name="es16")
        es16c3 = keep.tile([17, BLOC], f16, name="es16c3")
        nc.vector.memset(es16c3, 1.0)              # row 16 = b_lstm fold row
        ee16 = keep.tile([128, 3, BLOC], f16, name="ee16")
        ee16c3 = keep.tile([16, BLOC], f16, name="ee16c3")
        def load_E(b, defer=None):
            est = stage.tile([128, 3, L], f32, tag="estg", bufs=2)
            for c in range(3):
                nc.sync.dma_start(est[:, c, :], enc[b, 128 * c:128 * (c + 1), :])
            et = epool.tile([128, 3, L], f16, name=f"E16_{b}")
            if defer is None:
                nc.vector.tensor_copy(et, est)
            else:
                # cast deferred into sweep-s1 fill steps on the scalar queue
                # (keeps the pre-sweep vector queue clear for the reduces)
                defer.append(lambda et=et, est=est:
                             nc.scalar.activation(et, est, AF.Copy))
            E16.append(et)
            # initial es = E[:, 0, :] straight from the f32 stage (doesn't
            # wait on the big cast)
            nc.vector.tensor_copy(es16[:, :, b:b + 1], est[:, :, 0:1])
            est2 = stage.tile([128, 1600], f32, tag="stg")
            dma_eng2 = nc.scalar if b % 2 else nc.sync
            dma_eng2.dma_start(est2[:16, :L], enc[b, 384:400, :])
            ec3 = epool.tile([18, L], f16, name=f"E16c3_{b}")
            nc.vector.memset(ec3, 1.0)
            if defer is None:
                nc.vector.tensor_copy(ec3[:16], est2[:16, :L])
            else:
                defer.append(lambda ec3=ec3, est2=est2:
                             nc.scalar.activation(ec3[:16], est2[:16, :L], AF.Copy))
            nc.vector.tensor_copy(es16c3[:16, b:b + 1], est2[:16, 0:1])
            E16c3.append(ec3)

        # batches 0,1 first; 2,3 after the s-weights so the first sweep's
        # weight DMAs aren't starved behind all 6.5MB of E traffic
        load_E(0)
        load_E(1)

        # ---------------- weights ----------------
        # order: s-scoring weights first (first sweep waits only on these).
        # tag "s" casts: scalar (sweep weights) + vector (serial weights);
        # tag "e" + lstm casts: gpsimd (idle during sweep s1, done by gap 1)
        W = {}

        def cast(eng, dst, src):
            if eng is nc.scalar:
                nc.scalar.activation(dst, src, AF.Copy)
            else:
                eng.tensor_copy(dst, src)

        def load_scoring(tag, dma, csw, cse, stg):
            # generator: yields after each dma+cast step so the caller can
            # interleave emission with sweep instructions
            # stage-1 rhs rows 0..383 as fp16 [128, 3, 1600]
            w1m = wpool.tile([128, 3, H * PMX], f16, name=f"w1m_{tag}")
            for c in range(3):
                wst = stage.tile([128, 1600], f32, tag=stg, bufs=2)
                dma.dma_start(wst, wt[f"w1_{tag}"][128 * c:128 * (c + 1)])
                cast(csw, w1m[:, c, :], wst)
                yield
            W[f"w1m_{tag}"] = w1m
            # shared c3 rhs: rows 0..15 = W1[384:400] fp16, rows 16/17 = o hi/lo
            # (double-buffered by batch parity to relax WAR stalls)
            wst = stage.tile([128, 1600], f32, tag=stg, bufs=2)
            dma.dma_start(wst[:16], wt[f"w1_{tag}"][384:400])
            c3pair = []
            for pi in range(2):
                c3 = wpool.tile([18, H * PMX], f16, name=f"c3_{tag}_{pi}")
                cast(csw, c3[:16], wst[:16])
                c3pair.append(c3)
            yield
            W[f"c3_{tag}"] = c3pair
            # o-matmul rhs (fp16): rows 400..599 of w1, + b1 as ones-row 72 of c2
            wst = stage.tile([128, 1600], f32, tag=stg, bufs=2)
            dma.dma_start(wst, wt[f"w1_{tag}"][400:528])
            w1rc1 = wpool.tile([128, H * PMX], f16, name=f"w1r16c1_{tag}")
            cast(cse, w1rc1, wst)
            yield
            wst = stage.tile([128, 1600], f32, tag=stg, bufs=2)
            dma.dma_start(wst[:72], wt[f"w1_{tag}"][528:600])
            dma.dma_start(wst[72:73], wt[f"b1_{tag}"])
            w1rc2 = wpool.tile([73, H * PMX], f16, name=f"w1r16c2_{tag}")
            cast(cse, w1rc2, wst[:73])
            yield
            W[f"w1r_{tag}"] = (w1rc1, w1rc2)
            # wd fp16 k-chunk tiles over cat(hx[0:200], es[200:600], ee[600:1000])
            wd_chunks = []
            for (k0, kn) in [(0, 128), (128, 72), (200, 128), (328, 128), (456, 128), (584, 16),
                             (600, 128), (728, 128), (856, 128), (984, 16)]:
                wst = stage.tile([128, 1600], f32, tag=stg, bufs=2)
                dma.dma_start(wst[:kn, :H], wt[f"wd_{tag}"][k0:k0 + kn])
                t = wpool.tile([kn, H], f16, name=f"wd16_{tag}_{k0}")
                cast(cse, t, wst[:kn, :H])
                wd_chunks.append((k0, kn, t))
                yield
            W[f"wd_{tag}"] = wd_chunks

        def load_scoring2(tag, dma, csw, stg):
            # stage-2 rhs
            w2c1 = wpool.tile([128, H * PMX], f16, name=f"w2c1_{tag}")
            wst = stage.tile([128, 1600], f32, tag=stg, bufs=2)
            dma.dma_start(wst, wt[f"w2_{tag}"][:128])
            cast(csw, w2c1, wst)
            yield
            W[f"w2c1_{tag}"] = w2c1
            w2c2 = wpool.tile([74, H * PMX], f16, name=f"w2c2_{tag}")
            wst = stage.tile([128, 1600], f32, tag=stg, bufs=2)
            dma.dma_start(wst[:72], wt[f"w2_{tag}"][128:200])
            cast(csw, w2c2[:72], wst[:72])
            dma.dma_start(w2c2[72:73, :], wt[f"b2h_{tag}"])
            dma.dma_start(w2c2[73:74, :], wt[f"b2l_{tag}"])
            yield
            W[f"w2c2_{tag}"] = w2c2
            # stage-3 rhs chunks
            wst = stage.tile([128, 1600], f32, tag=stg, bufs=2)
            dma.dma_start(wst[:, 0:8], wt[f"w3_{tag}"][0:128])
            dma.dma_start(wst[:72, 8:16], wt[f"w3_{tag}"][128:200])
            dma.dma_start(wst[:, 16:24], wt[f"w3_{tag}"][200:328])
            dma.dma_start(wst[:72, 24:32], wt[f"w3_{tag}"][328:400])
            w3c1 = wpool.tile([128, PMX], f16, name=f"w3c1_{tag}")
            cast(csw, w3c1, wst[:, 0:8])
            w3c2 = wpool.tile([74, PMX], f16, name=f"w3c2_{tag}")
            cast(csw, w3c2[:72], wst[:72, 8:16])
            yield
            dma.dma_start(w3c2[72:73, :], wt[f"b3h_{tag}"])
            dma.dma_start(w3c2[73:74, :], wt[f"b3l_{tag}"])
            w3c3 = wpool.tile([128, PMX], f16, name=f"w3c3_{tag}")
            cast(csw, w3c3, wst[:, 16:24])
            w3c4 = wpool.tile([72, PMX], f16, name=f"w3c4_{tag}")
            cast(csw, w3c4, wst[:72, 24:32])
            yield
            W[f"w3_{tag}"] = (w3c1, w3c2, w3c3, w3c4)

        def load_lstm(dma, cse):
            # LSTM weights as fp16 rhs [kn, 800] over rows = cat(es, ee);
            # es-c3 chunk carries b_lstm as ones-row 16
            ih_chunks = []
            for (k0, kn) in [(0, 128), (128, 128), (256, 128), (384, 16),
                             (400, 128), (528, 128), (656, 128), (784, 16)]:
                wst = stage.tile([128, 1600], f32, tag="wstg_e", bufs=2)
                dma.dma_start(wst[:kn, :800], wt["w_ih"][k0:k0 + kn])
                rows = kn + 1 if k0 == 384 else kn
                t = wpool.tile([rows, 4 * H], f16, name=f"wih16_{k0}")
                if k0 == 384:
                    dma.dma_start(wst[16:17, :800], wt["b_lstm"])
                cast(cse, t[:rows], wst[:rows, :800])
                ih_chunks.append((k0, kn, t))
                yield
            # mlp fp16 rhs [kn, 200]; c2 carries b_mlp as ones-row 72
            wst = stage.tile([128, 1600], f32, tag="wstg_e", bufs=2)
            dma.dma_start(wst[:, :H], wt["w_mlp"][0:128])
            wmlpc1 = wpool.tile([128, H], f16, name="wmlp16c1")
            cast(cse, wmlpc1, wst[:, :H])
            yield
            wst = stage.tile([128, 1600], f32, tag="wstg_e", bufs=2)
            dma.dma_start(wst[:72, :H], wt["w_mlp"][128:200])
            dma.dma_start(wst[72:73, :H], wt["b_mlp"])
            wmlpc2 = wpool.tile([73, H], f16, name="wmlp16c2")
            cast(cse, wmlpc2, wst[:73, :H])
            W["ih_chunks"] = ih_chunks
            W["wmlp"] = (wmlpc1, wmlpc2)
            yield

        # s-tag weights emitted eagerly (first sweep waits on them); e-tag +
        # lstm weights emitted as fill steps interleaved into sweep s1
        import itertools
        for _ in load_scoring("s", nc.sync, nc.scalar, nc.vector, "stg"):
            pass
        for _ in load_scoring2("s", nc.sync, nc.scalar, "stg"):
            pass
        edefer = []
        load_E(2, edefer)
        load_E(3, edefer)

        def edefer_gen():
            for fn in edefer:
                fn()
                yield
        fill_steps = itertools.chain(
            edefer_gen(),
            load_scoring("e", nc.sync, nc.scalar, nc.scalar, "wstg_e"),
            load_scoring2("e", nc.sync, nc.scalar, "wstg_e"),
            load_lstm(nc.sync, nc.scalar))

        # ---------------- persistent state tiles ----------------
        hxT = [keep.tile([128, BLOC], f16, name="hxT16_0"),
               keep.tile([72, BLOC], f16, name="hxT16_1")]
        rT1 = keep.tile([128, BLOC], f16, name="rT1")
        rT2 = keep.tile([73, BLOC], f16, name="rT2")
        nc.gpsimd.memset(rT2, 1.0)                 # row 72 = b1 fold row
        h0T1 = keep.tile([128, BLOC], f16, name="h0T1")
        h0T2 = keep.tile([73, BLOC], f16, name="h0T2")
        nc.gpsimd.memset(h0T2, 1.0)                # row 72 = b_mlp fold row

        # es/ee init happened inline in the E load loop (from the f32 stage)
        nc.vector.tensor_copy(ee16, es16)
        nc.vector.tensor_copy(ee16c3, es16c3[:16])

        # manually rotated work slots; m1c2 slots carry persistent ones rows
        # 72/73 (paired with the b2/b3 hi+lo rhs rows)
        m1_slots, m2_slots, m1c2_slots = [], [], []
        for i in range(6):
            m1_slots.append(keep.tile([128, H], f16, name=f"m1_slot{i}"))
            m2_slots.append(keep.tile([128, H], f16, name=f"m2_slot{i}"))
            t = keep.tile([74, 128], f16, name=f"m1c2_slot{i}")
            nc.gpsimd.memset(t, 1.0)
            m1c2_slots.append(t)
        S4_a = keep.tile([BLOC, L], f32, name="S4_a")
        S4_b = keep.tile([BLOC, L], f32, name="S4_b")

        dma_engines = [nc.sync, nc.scalar, nc.gpsimd]

        # ---------------- helpers ----------------
        def cat_chunks(tag, with_hx):
            """(lhsT [kn,4] fp16, wd16 [kn,H]) pairs for r = tanh(cat @ wd)."""
            ops = []
            for (k0, kn, wtile) in W[f"wd_{tag}"]:
                if k0 < 200:
                    if not with_hx:
                        continue
                    lhsT = hxT[0] if k0 == 0 else hxT[1]
                elif k0 < 600:
                    c = (k0 - 200) // 128
                    lhsT = es16[:, c, :] if c < 3 else es16c3[:16]
                else:
                    c = (k0 - 600) // 128
                    lhsT = ee16[:, c, :] if c < 3 else ee16c3
                ops.append((lhsT, wtile))
            return ops

        def r_matmul(tag, with_hx):
            """r_row = tanh(cat @ wd) -> [4, H] fp16 sbuf."""
            ops = cat_chunks(tag, with_hx)
            pt = ps_d.tile([128, 512], f32, tag="ps_ser")
            for i, (lhsT, rhs) in enumerate(ops):
                nc.tensor.matmul(pt[:BLOC, :H], lhsT, rhs,
                                 start=(i == 0), stop=(i == len(ops) - 1))
            r_row = work.tile([BLOC, H], f16, tag="r_row", bufs=1)
            nc.scalar.activation(r_row, pt[:BLOC, :H], AF.Tanh)
            return r_row

        def o_rows(tag, r_row):
            """o = r@w1r + b1 (f32 psum) -> fp16 hi/lo rows [BLOC, 1600]."""
            # transpose r to column chunks [128,4], [72,4] (+ones row 72)
            ptr = ps_tr.tile([128, 128], f16, tag="ps_tr")
            nc.tensor.transpose(ptr[:, :BLOC], r_row[:, 0:128], ident[:BLOC, :BLOC])
            nc.vector.tensor_copy(rT1, ptr[:, :BLOC])
            ptr2 = ps_tr.tile([128, 128], f16, tag="ps_tr")
            nc.tensor.transpose(ptr2[:72, :BLOC], r_row[:, 128:200], ident[:BLOC, :BLOC])
            nc.vector.tensor_copy(rT2[:72], ptr2[:72, :BLOC])
            w1rc1, w1rc2 = W[f"w1r_{tag}"]
            oh = single.tile([BLOC, H * PMX], f16, tag="oh")
            ol = single.tile([BLOC, H * PMX], f16, tag="ol")
            for (n0, nn) in NCH:
                pt = ps_d.tile([128, 512], f32, tag="ps_ser")
                nc.tensor.matmul(pt[:BLOC, :nn], rT1, w1rc1[:, n0:n0 + nn], start=True, stop=False)
                nc.tensor.matmul(pt[:BLOC, :nn], rT2, w1rc2[:, n0:n0 + nn], start=False, stop=True)
                nc.scalar.activation(oh[:, n0:n0 + nn], pt[:BLOC, :nn], AF.Copy)
                nc.vector.tensor_tensor(ol[:, n0:n0 + nn], pt[:BLOC, :nn], oh[:, n0:n0 + nn], OP.subtract)
            return oh, ol

        def score_sweep(tag, S4, oh, ol, fill=None):
            """Maxout scoring, software-pipelined across (b, lt) tiles."""
            w1m = W[f"w1m_{tag}"]
            c3pair = W[f"c3_{tag}"]
            w2c1 = W[f"w2c1_{tag}"]
            w2c2 = W[f"w2c2_{tag}"]
            w3c1, w3c2, w3c3, w3c4 = W[f"w3_{tag}"]
            NT = BLOC * NLT
            st = [dict() for _ in range(NT)]
            strips = {}

            def g1(i):
                b, lt = divmod(i, NLT)
                c3rhs = c3pair[b % 2]
                if lt == 0:
                    nc.sync.dma_start(c3rhs[16:17, :], oh[b:b + 1, :])
                    nc.sync.dma_start(c3rhs[17:18, :], ol[b:b + 1, :])
                lsl = slice(128 * lt, 128 * (lt + 1))
                m1 = m1_slots[i % 6]
                for ni, (n0, nn) in enumerate(NCH):
                    pa = ps_a.tile([128, 512], f32, tag="ps_s1")
                    for c in range(3):
                        nc.tensor.matmul(pa[:, :nn], E16[b][:, c, lsl], w1m[:, c, n0:n0 + nn],
                                         start=(c == 0), stop=False)
                    nc.tensor.matmul(pa[:, :nn], E16c3[b][:, lsl], c3rhs[:, n0:n0 + nn],
                                     start=False, stop=True)
                    h0, hn = HSL[ni]
                    nc.vector.tensor_reduce(
                        m1[:, h0:h0 + hn],
                        pa[:, :nn].rearrange("p (h q) -> p h q", q=PMX),
                        axis=AX.X, op=OP.max)
                st[i]["m1"] = m1

            def g2(i):
                m1 = st[i]["m1"]
                pt1 = ps_tr.tile([128, 128], f16, tag="ps_tr")
                nc.tensor.transpose(pt1, m1[:, 0:128], ident)
                m1c1 = work.tile([128, 128], f16, tag="m1c1", bufs=4)
                nc.scalar.activation(m1c1, pt1, AF.Copy)
                pt2 = ps_tr.tile([128, 128], f16, tag="ps_tr")
                nc.tensor.transpose(pt2[:72], m1[:, 128:200], ident)
                m1c2 = m1c2_slots[i % 6]
                nc.scalar.activation(m1c2[:72], pt2[:72], AF.Copy)
                m2 = m2_slots[i % 6]
                for ni, (n0, nn) in enumerate(NCH):
                    pb = ps_b.tile([128, 512], f32, tag="ps_s2")
                    nc.tensor.matmul(pb[:, :nn], m1c1, w2c1[:, n0:n0 + nn], start=True, stop=False)
                    nc.tensor.matmul(pb[:, :nn], m1c2, w2c2[:, n0:n0 + nn], start=False, stop=True)
                    h0, hn = HSL[ni]
                    nc.vector.tensor_reduce(
                        m2[:, h0:h0 + hn],
                        pb[:, :nn].rearrange("p (h q) -> p h q", q=PMX),
                        axis=AX.X, op=OP.max)
                st[i]["m1c1"] = m1c1
                st[i]["m1c2"] = m1c2
                st[i]["m2"] = m2

            def g3(i):
                b, lt = divmod(i, NLT)
                m2 = st[i]["m2"]
                pt3 = ps_tr.tile([128, 128], f16, tag="ps_tr")
                nc.tensor.transpose(pt3, m2[:, 0:128], ident)
                m2c1 = work.tile([128, 128], f16, tag="m2c1", bufs=4)
                nc.scalar.activation(m2c1, pt3, AF.Copy)
                pt4 = ps_tr.tile([128, 128], f16, tag="ps_tr")
                nc.tensor.transpose(pt4[:72], m2[:, 128:200], ident)
                m2c2 = work.tile([72, 128], f16, tag="m2c2", bufs=4)
                nc.scalar.activation(m2c2, pt4[:72], AF.Copy)
                if lt == 0:
                    strips[b] = ps_d.tile([128, 8 * NLT], f32, tag="ps_ser", name="s3strip")
                psl = strips[b][:, 8 * lt:8 * (lt + 1)]
                nc.tensor.matmul(psl, st[i]["m1c1"], w3c1, start=True, stop=False)
                nc.tensor.matmul(psl, st[i]["m1c2"], w3c2, start=False, stop=False)
                nc.tensor.matmul(psl, m2c1, w3c3, start=False, stop=False)
                nc.tensor.matmul(psl, m2c2, w3c4, start=False, stop=True)
                st[i].clear()
                if lt == NLT - 1:
                    Sb = work.tile([128, NLT], f32, tag="Sb")
                    nc.vector.tensor_reduce(Sb,
                                            strips[b].rearrange("p (t q) -> p t q", q=PMX),
                                            axis=AX.X, op=OP.max)
                    ptb = ps_tr.tile([NLT, 128], f32, tag="ps_tr")
                    nc.tensor.transpose(ptb, Sb, ident32)
                    s4stg = work.tile([NLT, 128], f32, tag="s4stg")
                    nc.scalar.activation(s4stg, ptb, AF.Copy)
                    dma_engines[b % 3].dma_start(S4[b:b + 1, :], s4stg)
            for i in range(NT + 2):
                if i < NT:
                    g1(i)
                if 1 <= i < NT + 1:
                    g2(i - 1)
                if 2 <= i:
                    g3(i - 2)
                if fill is not None:
                    next(fill, None)
            if fill is not None:
                for _ in fill:
                    pass
            nc.vector.tensor_tensor(S4, S4, pen_row, OP.subtract)

        def argmax_gather(S4, dstbig, dstc3):
            """argmax over S4 rows; gather E columns (fp16)."""
            mx8 = work.tile([BLOC, 8], f32, tag="mx8")
            idx8 = work.tile([BLOC, 8], u32, tag="idx8")
            nc.vector.max(out=mx8, in_=S4)
            nc.vector.max_index(out=idx8, in_max=mx8, in_values=S4)
            for b in range(BLOC):
                reg = nc.values_load(idx8[b:b + 1, 0:1], min_val=0, max_val=L - 1,
                                     skip_runtime_bounds_check=True)
                dma_engines[(2 * b) % 3].dma_start(
                    dstbig[:, :, b:b + 1], E16[b][:, :, ds(reg, 1)])
                dma_engines[(2 * b + 1) % 3].dma_start(
                    dstc3[:16, b:b + 1], E16c3[b][:16, ds(reg, 1)])

        def lstm_update():
            """hx via LSTM cell with hx0=cx0=0 (f-gate and w_hh drop out)."""
            # gates row-layout: psum [4, 200] for i; [4, 400] for g,o
            pt_i = ps_d.tile([128, 512], f32, tag="ps_ser")
            pt_go = ps_d.tile([128, 512], f32, tag="ps_ser")
            lhs_for = []
            for (k0, kn, wtile) in W["ih_chunks"]:
                if k0 < 400:
                    c = k0 // 128
                    lhsT = es16[:, c, :] if c < 3 else es16c3  # [17,4] w/ ones
                else:
                    c = (k0 - 400) // 128
                    lhsT = ee16[:, c, :] if c < 3 else ee16c3
                lhs_for.append((lhsT, wtile, kn + (1 if k0 == 384 else 0)))
            n = len(lhs_for)
            for i, (lhsT, wtile, rows) in enumerate(lhs_for):
                nc.tensor.matmul(pt_i[:BLOC, :H], lhsT, wtile[:rows, 0:H],
                                 start=(i == 0), stop=(i == n - 1))
            for i, (lhsT, wtile, rows) in enumerate(lhs_for):
                nc.tensor.matmul(pt_go[:BLOC, :2 * H], lhsT, wtile[:rows, 2 * H:4 * H],
                                 start=(i == 0), stop=(i == n - 1))
            ig = work.tile([BLOC, H], f32, tag="ig", bufs=1)
            nc.scalar.activation(ig, pt_i[:BLOC, :H], AF.Sigmoid)
            gg = work.tile([BLOC, H], f32, tag="gg", bufs=1)
            nc.scalar.activation(gg, pt_go[:BLOC, 0:H], AF.Tanh)
            og = work.tile([BLOC, H], f32, tag="og", bufs=1)
            nc.scalar.activation(og, pt_go[:BLOC, H:2 * H], AF.Sigmoid)
            cx = work.tile([BLOC, H], f32, tag="cx", bufs=1)
            nc.vector.tensor_tensor(cx, ig, gg, OP.mult)
            tcx = work.tile([BLOC, H], f32, tag="tcx", bufs=1)
            nc.scalar.activation(tcx, cx, AF.Tanh)
            h0 = work.tile([BLOC, H], f16, tag="h0", bufs=1)
            nc.vector.tensor_tensor(h0, og, tcx, OP.mult)
            # transpose h0 -> column chunks (+ones row 72 for b_mlp)
            ptr = ps_tr.tile([128, 128], f16, tag="ps_tr")
            nc.tensor.transpose(ptr[:, :BLOC], h0[:, 0:128], ident[:BLOC, :BLOC])
            nc.vector.tensor_copy(h0T1, ptr[:, :BLOC])
            ptr2 = ps_tr.tile([128, 128], f16, tag="ps_tr")
            nc.tensor.transpose(ptr2[:72, :BLOC], h0[:, 128:200], ident[:BLOC, :BLOC])
            nc.vector.tensor_copy(h0T2[:72], ptr2[:72, :BLOC])
            # mlp: hx = h0 @ w_mlp + b_mlp
            pt = ps_d.tile([128, 512], f32, tag="ps_ser")
            wmlpc1, wmlpc2 = W["wmlp"]
            nc.tensor.matmul(pt[:BLOC, :H], h0T1, wmlpc1, start=True, stop=False)
            nc.tensor.matmul(pt[:BLOC, :H], h0T2, wmlpc2, start=False, stop=True)
            hx_row = work.tile([BLOC, H], f16, tag="hx_row", bufs=1)
            nc.scalar.activation(hx_row, pt[:BLOC, :H], AF.Copy)
            # transpose to hxT chunks
            ptr3 = ps_tr.tile([128, 128], f16, tag="ps_tr")
            nc.tensor.transpose(ptr3[:, :BLOC], hx_row[:, 0:128], ident[:BLOC, :BLOC])
            nc.vector.tensor_copy(hxT[0], ptr3[:, :BLOC])
            ptr4 = ps_tr.tile([128, 128], f16, tag="ps_tr")
            nc.tensor.transpose(ptr4[:72, :BLOC], hx_row[:, 128:200], ident[:BLOC, :BLOC])
            nc.vector.tensor_copy(hxT[1], ptr4[:72, :BLOC])

        def log_softmax_out(S4, out_dram):
            gmax = work.tile([BLOC, 1], f32, tag="gmax")
            nc.vector.tensor_reduce(gmax, S4, axis=AX.X, op=OP.max)
            negm = work.tile([BLOC, 1], f32, tag="negm")
            nc.vector.tensor_scalar_mul(negm, gmax, -1.0)
            e4 = single.tile([BLOC, L], f32, tag="e4")
            sume = work.tile([BLOC, 1], f32, tag="sume")
            nc.scalar.activation(e4, S4, AF.Exp, bias=negm[:, 0:1], accum_out=sume)
            lnz = work.tile([BLOC, 1], f32, tag="lnz")
            nc.scalar.activation(lnz, sume, AF.Ln)
            lse = work.tile([BLOC, 1], f32, tag="lse")
            nc.vector.tensor_tensor(lse, gmax, lnz, OP.add)
            lp4 = single.tile([BLOC, L], f32, tag="e4")
            nc.vector.tensor_scalar(lp4, S4, lse[:, 0:1], None, op0=OP.subtract)
            nc.sync.dma_start(out_dram, lp4)

        # ---------------- the four passes ----------------
        r_row = r_matmul("s", with_hx=False)
        oh, ol = o_rows("s", r_row)
        S4_t0 = work.tile([BLOC, L], f32, tag="S4_tmp", bufs=2)
        score_sweep("s", S4_t0, oh, ol, fill=fill_steps)
        argmax_gather(S4_t0, es16, es16c3)

        r_row = r_matmul("e", with_hx=False)
        oh, ol = o_rows("e", r_row)
        S4_t1 = work.tile([BLOC, L], f32, tag="S4_tmp", bufs=2)
        score_sweep("e", S4_t1, oh, ol)
        argmax_gather(S4_t1, ee16, ee16c3)

        lstm_update()

        r_row = r_matmul("s", with_hx=True)
        oh, ol = o_rows("s", r_row)
        score_sweep("s", S4_a, oh, ol)
        argmax_gather(S4_a, es16, es16c3)

        # issue e2's serial path before lp1's log_softmax so the (vector/
        # scalar) softmax chain doesn't delay the e2 sweep start; lsm then
        # overlaps the e2 sweep
        r_row = r_matmul("e", with_hx=True)
        oh, ol = o_rows("e", r_row)
        log_softmax_out(S4_a, lp1)
        score_sweep("e", S4_b, oh, ol)
        log_softmax_out(S4_b, lp2)

        stage_cm.__exit__(None, None, None)

    nc.compile()
    return nc


def get_program():
    if "nc" not in _cache:
        _cache["nc"] = _build_program()
    return _cache["nc"]


def _split16(x):
    hi = np.asarray(x, np.float32).astype(np.float16)
    lo = (np.asarray(x, np.float32) - hi.astype(np.float32)).astype(np.float16)
    return hi, lo


def make_in_maps(inputs):
    """Per-core input maps: batch shard + trivial host prep (mask, bias splits)."""
    inputs = {k: np.asarray(v) for k, v in inputs.items()}
    enc = np.ascontiguousarray(inputs["encoding_matrix"], dtype=np.float32)
    lens = np.asarray(inputs["passage_lens"]).astype(np.int64)
    pen_full = np.where(np.arange(L)[None, :] < lens[:, None],
                        np.float32(0.0), BIG).astype(np.float32)

    shared = {}
    for tag in ("s", "e"):
        shared[f"w1_{tag}"] = np.ascontiguousarray(inputs[f"w1_{tag}"], np.float32)
        shared[f"b1_{tag}"] = np.ascontiguousarray(inputs[f"b1_{tag}"], np.float32).reshape(1, -1)
        shared[f"w2_{tag}"] = np.ascontiguousarray(inputs[f"w2_{tag}"], np.float32)
        b2h, b2l = _split16(inputs[f"b2_{tag}"])
        shared[f"b2h_{tag}"] = b2h.reshape(1, -1)
        shared[f"b2l_{tag}"] = b2l.reshape(1, -1)
        shared[f"w3_{tag}"] = np.ascontiguousarray(inputs[f"w3_{tag}"], np.float32)
        b3h, b3l = _split16(inputs[f"b3_{tag}"])
        shared[f"b3h_{tag}"] = b3h.reshape(1, -1)
        shared[f"b3l_{tag}"] = b3l.reshape(1, -1)
        shared[f"wd_{tag}"] = np.ascontiguousarray(inputs[f"wd_{tag}"], np.float32)
    shared["w_ih"] = np.ascontiguousarray(inputs["w_ih"], np.float32)
    shared["b_lstm"] = np.ascontiguousarray(inputs["b_lstm"], np.float32).reshape(1, -1)
    shared["w_mlp"] = np.ascontiguousarray(inputs["w_mlp"], np.float32)
    shared["b_mlp"] = np.ascontiguousarray(inputs["b_mlp"], np.float32).reshape(1, -1)

    in_maps = []
    for core in range(NCORES):
        sl = slice(core * BLOC, (core + 1) * BLOC)
        m = dict(shared)
        m["enc"] = np.ascontiguousarray(enc[sl])
        m["pen"] = np.ascontiguousarray(pen_full[sl])
        in_maps.append(m)
    return in_maps


def run_on_hw(inputs, trace=False):
    from concourse import bass_utils
    nc = get_program()
    in_maps = make_in_maps(inputs)
    res = bass_utils.run_bass_kernel_spmd(nc, in_maps, core_ids=list(range(NCORES)),
                                          trace=trace)
    lp1 = np.concatenate([res.results[c]["lp1"] for c in range(NCORES)], axis=0)
    lp2 = np.concatenate([res.results[c]["lp2"] for c in range(NCORES)], axis=0)
    return (np.asarray(lp1, np.float32), np.asarray(lp2, np.float32)), res


def kernel(**inputs):
    out, _ = run_on_hw(inputs, trace=False)
    return out
